# revision 14
# baseline (speedup 1.0000x reference)
"""BiLSTM-CRF Viterbi decode on 8 Trainium2 NeuronCores (Bass/Tile).

Strategy (self-contained, shapes hardcoded):
  - 8 cores, core c owns sentence positions [512c, 512(c+1)).
  - Embedding table replicated; each core indirect-DMA-gathers the 640
    token rows covering [512c-64, 512c+576) (out-of-range -> zero rows).
  - pre = x @ w_ih.T + b precomputed on PE for both directions.
  - LSTM recurrence: chunked with zero-init warmup. Per core two
    interleaved streams: forward (128 chunks x 4 tokens, warmup 48) and
    backward (136 chunks covering [512c-32, 512(c+1)), warmup 48).
    Gates are computed on PE (fp32), sigmoid via 0.5*tanh(x/2)+0.5 (the
    Tanh LUT is ~1 ulp), cell update on DVE. Warmup makes each chunk's
    state agree with the sequential scan to ~1 ulp (contraction of the
    LSTM map); boundary cores are exact via zero-x padding.
  - feats = h @ w_tag.T + b_tag computed directly in [chunk, (t, tag)]
    layout (144 small matmuls with column-strided stationary operands).
  - Viterbi: 128 chunks x 4 positions per core, warmup 32 from vinit
    (max-plus coalescence); core-0 short chunks are made exact with
    per-step copy_predicated restores. Backpointers recovered in one
    batched pass (is_equal + reversed-iota + max-reduce).
  - Host: backtrace over device backpointers, score re-accumulated in
    fp32 along the path (bitwise-faithful to the reference recursion).
"""
import os
import sys

for _p in ("/opt/trn_rl_repo", "/root/.axon_site/_ro/trn_rl_repo"):
    if os.path.isdir(_p) and _p not in sys.path:
        sys.path.append(_p)

import numpy as np

import concourse.bass as bass
import concourse.bacc as bacc
import concourse.mybir as mybir
import concourse.tile as tile
from concourse.bass_utils import run_bass_kernel_spmd
from concourse.masks import make_identity

f32 = mybir.dt.float32
i32 = mybir.dt.int32

# problem constants
S = 4096
V = 100000
E = 300
HH = 256
G4 = 1024
T = 24
START, STOP = 22, 23
NEG = -10000.0

NCORES = 8
SPAN = S // NCORES          # 512 positions per core
L = 4                       # tokens per LSTM/viterbi chunk
W = 36                      # LSTM warmup steps
WBF = 20                    # of which: bf16 hh-matmul head
WV = 24                     # viterbi warmup steps
BF = SPAN // L              # 128 forward chunks
BB = (SPAN + WV) // L       # 136 backward chunks (cover 32-pos left spill)
NSTEP = L + W               # 52 LSTM steps per stream
VSTEP = L + WV              # 36 viterbi steps
NPOS = 640                  # gathered x / pre positions: [512c-64, 512c+576)
XOFF = 64                   # position p -> pre column p - 512c + XOFF
HFULL = SPAN + WV           # 544 h columns: [512c-32, 512(c+1))
OOB = 1 << 20

_PROG_CACHE = {}


def _build_program():
    nc = bacc.Bacc("TRN2", target_bir_lowering=False, debug=False,
                   num_devices=NCORES)
    P = 128

    def din(name, shape, dt=f32):
        return nc.dram_tensor(name, shape, dt, kind="ExternalInput").ap()

    def dout(name, shape, dt=f32):
        return nc.dram_tensor(name, shape, dt, kind="ExternalOutput").ap()

    emb = din("emb", [V, E])
    tok = din("tok", [NPOS], i32)
    wihT = din("wihT", [2, 3, P, G4])       # [dir, ktile, 128, 1024] (E padded to 384)
    whhT = din("whhT", [2, 2, P, G4])
    biasr = din("biasr", [2, 8, P, 1])
    wtagT = din("wtagT", [4, P, T])
    btag_rep = din("btag_rep", [P, VSTEP * T])
    transR = din("transR", [P, T * T])
    vmask = din("vmask", [P, WV], i32)
    iot = din("iot", [P, L * T * T])
    vinit_rep = din("vinit_rep", [P, T])

    o_bptr = dout("bptrs", [P, L * T])
    o_vhist = dout("vhist", [P, VSTEP * T])
    o_feats = dout("featsv", [P, VSTEP * T])

    def bc_mid(ap2, n):
        return bass.AP(ap2.tensor, ap2.offset, [ap2.ap[0], [0, n], ap2.ap[1]])

    def bc_last(ap2, n):
        return bass.AP(ap2.tensor, ap2.offset, [ap2.ap[0], ap2.ap[1], [0, n]])

    with tile.TileContext(nc) as tc:
        with (
            tc.tile_pool(name="const", bufs=1) as cpool,
            tc.tile_pool(name="state", bufs=1) as spool,
            tc.tile_pool(name="vit", bufs=1) as vpool,
        ):
            # ---- constants into SBUF ----
            ident = cpool.tile([P, P], f32)
            make_identity(nc, ident[:])
            whh_sb = {}
            for d in range(2):
                for k in range(2):
                    t_ = cpool.tile([P, G4], f32, tag=f"whh{d}{k}", name=f"whh{d}{k}")
                    nc.sync.dma_start(out=t_[:], in_=whhT[d, k])
                    whh_sb[d, k] = t_
            wtag_sb = []
            for k in range(4):
                t_ = cpool.tile([P, T], f32, tag=f"wtag{k}", name=f"wtag{k}")
                nc.sync.dma_start(out=t_[:], in_=wtagT[k])
                wtag_sb.append(t_)
            bias_sb = cpool.tile([P, 16], f32)
            for d in range(2):
                for m in range(8):
                    nc.sync.dma_start(out=bias_sb[:, d * 8 + m:d * 8 + m + 1],
                                      in_=biasr[d, m])
            btag_sb = vpool.tile([P, VSTEP * T], f32)
            nc.sync.dma_start(out=btag_sb[:], in_=btag_rep[:])
            trR_sb = vpool.tile([P, T * T], f32)
            nc.sync.dma_start(out=trR_sb[:], in_=transR[:])
            vmask_sb = vpool.tile([P, WV], i32)
            nc.sync.dma_start(out=vmask_sb[:], in_=vmask[:])
            iot_sb = vpool.tile([P, L * T * T], f32)
            nc.sync.dma_start(out=iot_sb[:], in_=iot[:])
            vinit_sb = vpool.tile([P, T], f32)
            nc.sync.dma_start(out=vinit_sb[:], in_=vinit_rep[:])

            # persistent LSTM state / outputs
            hfull = {}
            for d in range(2):
                for k in range(2):
                    t_ = spool.tile([P, HFULL], f32, tag=f"hfull{d}{k}", name=f"hfull{d}{k}")
                    hfull[d, k] = t_
            pre_f = spool.tile([P, 8 * NPOS], f32, tag="pref", name="pref")
            pre_pair = {}
            for j in range(4):
                t_ = spool.tile([P, 2 * NPOS], f32, tag=f"pre1{j}", name=f"pre1{j}")
                pre_pair[1, j] = t_

            # ---- phase A: gather x, build x^T ----
            with (
                tc.tile_pool(name="phA", bufs=1) as apool,
                tc.tile_pool(name="psA", bufs=2, space="PSUM") as psA,
                nc.named_scope("gather_pre"),
            ):
                tok_sb = apool.tile([P, 5], i32)
                nc.sync.dma_start(out=tok_sb[:],
                                  in_=tok.rearrange("(j p) -> p j", p=P))
                xg = []
                for j in range(5):
                    xt = apool.tile([P, E], f32, tag=f"x{j}", name=f"x{j}")
                    nc.vector.memset(xt[:], 0.0)
                    nc.gpsimd.indirect_dma_start(
                        out=xt[:], out_offset=None, in_=emb[:, :],
                        in_offset=bass.IndirectOffsetOnAxis(
                            ap=tok_sb[:, j:j + 1], axis=0),
                        bounds_check=V - 1, oob_is_err=False)
                    xg.append(xt)
                xT = []
                for k in range(3):
                    t_ = apool.tile([P, NPOS], f32, tag=f"xT{k}", name=f"xT{k}")
                    if k == 2:
                        nc.vector.memset(t_[:], 0.0)
                    xT.append(t_)
                for j in range(5):
                    for k in range(3):
                        ecols = min(128, E - 128 * k)
                        tp = psA.tile([P, P], f32, space="PSUM", tag="tp", name="tp")
                        nc.tensor.transpose(tp[:ecols, :],
                                            xg[j][:, 128 * k:128 * k + ecols],
                                            ident[:])
                        nc.vector.tensor_copy(
                            xT[k][:ecols, j * P:(j + 1) * P], tp[:ecols, :])

                # ---- phase B: pre = x @ w_ih^T + b ----
                wih_sb = {}
                for d in range(2):
                    for k in range(3):
                        t_ = apool.tile([P, G4], f32, tag=f"wih{d}{k}", name=f"wih{d}{k}")
                        nc.sync.dma_start(out=t_[:], in_=wihT[d, k])
                        wih_sb[d, k] = t_
                with tc.tile_pool(name="psB", bufs=4, space="PSUM") as psB:
                    for d in range(2):
                        for m in range(8):
                            for ncol, c0 in ((512, 0), (128, 512)):
                                pm = psB.tile([P, 512], f32, space="PSUM",
                                              tag="pm", name="pm")
                                for k in range(3):
                                    nc.tensor.matmul(
                                        pm[:, :ncol],
                                        wih_sb[d, k][:, 128 * m:128 * (m + 1)],
                                        xT[k][:, c0:c0 + ncol],
                                        start=(k == 0), stop=(k == 2))
                                if d == 0:
                                    dst = pre_f[:, m * NPOS + c0:
                                                m * NPOS + c0 + ncol]
                                else:
                                    j, half = divmod(m, 2)
                                    dst = pre_pair[1, j][:, half * NPOS + c0:
                                                         half * NPOS + c0 + ncol]
                                nc.vector.tensor_scalar_add(
                                    dst, pm[:, :ncol],
                                    bias_sb[:, d * 8 + m:d * 8 + m + 1])

            # ---- phase C: LSTM streams ----
            streams = [
                dict(d=0, B=BF),   # forward: single [128, 8B] psum (2 banks)
                dict(d=1, B=BB),   # backward: 4 psum tensors [128, 2B]
            ]
            with tc.tile_pool(name="psC", bufs=1, space="PSUM") as psC, \
                    nc.named_scope("lstm"):
                for st in streams:
                    B = st["B"]
                    d = st["d"]
                    st["h"] = spool.tile([P, 2 * B], f32, tag=f"h{d}", name=f"hcur{d}")
                    st["c"] = spool.tile([P, 2 * B], f32, tag=f"c{d}", name=f"ccur{d}")
                    st["tmp"] = spool.tile([P, 6 * B], f32, tag=f"tm{d}", name=f"tmp{d}")
                    st["sig"] = spool.tile([P, 6 * B], f32, tag=f"sg{d}", name=f"sig{d}")
                    st["tg"] = spool.tile([P, 2 * B], f32, tag=f"tg{d}", name=f"tgg{d}")
                    st["tc"] = spool.tile([P, 2 * B], f32, tag=f"tc{d}", name=f"tcc{d}")
                    st["pr"] = spool.tile([P, 2 * B], f32, tag=f"pr{d}", name=f"prd{d}")
                    st["hbf"] = spool.tile([P, 2 * B], mybir.dt.bfloat16,
                                           tag=f"hb{d}", name=f"hbf{d}")
                    nc.vector.memset(st["h"][:], 0.0)
                    nc.vector.memset(st["c"][:], 0.0)
                    nc.vector.memset(st["hbf"][:], 0.0)
                whh_bf = {}
                for d in range(2):
                    for k in range(2):
                        t_ = spool.tile([P, G4], mybir.dt.bfloat16,
                                        tag=f"whb{d}{k}", name=f"whb{d}{k}")
                        nc.vector.tensor_copy(t_[:], whh_sb[d, k][:])
                        whh_bf[d, k] = t_
                ident_bf = spool.tile([P, P], mybir.dt.bfloat16, name="identbf")
                nc.vector.tensor_copy(ident_bf[:], ident[:])
                pre_f_bf = spool.tile([P, 8 * NPOS], mybir.dt.bfloat16,
                                      tag="prefbf", name="prefbf")
                nc.vector.tensor_copy(pre_f_bf[:], pre_f[:])
                pre_bf1 = {}
                for j in range(4):
                    t_ = spool.tile([P, 2 * NPOS], mybir.dt.bfloat16,
                                    tag=f"preb1{j}", name=f"preb1{j}")
                    nc.vector.tensor_copy(t_[:], pre_pair[1, j][:])
                    pre_bf1[j] = t_

                for t in range(NSTEP):
                    for st in streams:
                        d, B = st["d"], st["B"]
                        if d == 0:
                            poff = (XOFF - W) + t
                            ps = [psC.tile([P, 2 * B], f32, space="PSUM",
                                           tag=f"psf{j}", name=f"psf{j}")
                                  for j in range(4)]
                            bf = t < WBF
                            wsel = whh_bf if bf else whh_sb
                            hsel = st["hbf"] if bf else st["h"]
                            for j in range(4):
                                for half in range(2):
                                    m = 2 * j + half
                                    if bf:
                                        pslice = pre_f_bf[:, m * NPOS + poff::L][:, :B]
                                        nc.tensor.matmul(
                                            ps[j][:, half * B:(half + 1) * B],
                                            ident_bf[:], pslice,
                                            start=True, stop=False)
                                    for k in range(2):
                                        nc.tensor.matmul(
                                            ps[j][:, half * B:(half + 1) * B],
                                            wsel[0, k][:, 128 * m:128 * (m + 1)],
                                            hsel[:, k * B:(k + 1) * B],
                                            start=(k == 0 and not bf),
                                            stop=(k == 1))
                            if not bf:
                                for j in range(4):
                                    pslice = pre_f[:].rearrange(
                                        "p (m n) -> p m n", m=8)[:, 2 * j:2 * j + 2,
                                                                poff::L][:, :, :B]
                                    nc.vector.tensor_tensor(
                                        out=ps[j][:].rearrange("p (m b) -> p m b", m=2),
                                        in0=ps[j][:].rearrange("p (m b) -> p m b", m=2),
                                        in1=pslice, op=mybir.AluOpType.add)
                            for j in range(3):
                                nc.scalar.activation(
                                    st["tmp"][:, j * 2 * B:(j + 1) * 2 * B],
                                    ps[j][:],
                                    mybir.ActivationFunctionType.Tanh,
                                    bias=0.0, scale=0.5)
                            nc.scalar.activation(
                                st["sig"][:], st["tmp"][:],
                                mybir.ActivationFunctionType.Copy,
                                bias=0.5, scale=0.5)
                            nc.scalar.activation(
                                st["tg"][:], ps[3][:],
                                mybir.ActivationFunctionType.Tanh)
                        else:
                            poff = (XOFF - WV) + (L - 1) + W - t
                            ps = [psC.tile([P, 2 * B], f32, space="PSUM",
                                           tag=f"psb{j}", name=f"psb{j}")
                                  for j in range(4)]
                            bf = t < WBF
                            wsel = whh_bf if bf else whh_sb
                            hsel = st["hbf"] if bf else st["h"]
                            for j in range(4):
                                for half in range(2):
                                    m = 2 * j + half
                                    if bf:
                                        pslice = pre_bf1[j][:, half * NPOS + poff::L][:, :B]
                                        nc.tensor.matmul(
                                            ps[j][:, half * B:(half + 1) * B],
                                            ident_bf[:], pslice,
                                            start=True, stop=False)
                                    for k in range(2):
                                        nc.tensor.matmul(
                                            ps[j][:, half * B:(half + 1) * B],
                                            wsel[1, k][:, 128 * m:128 * (m + 1)],
                                            hsel[:, k * B:(k + 1) * B],
                                            start=(k == 0 and not bf),
                                            stop=(k == 1))
                            if not bf:
                                for j in range(4):
                                    pslice = pre_pair[1, j][:].rearrange(
                                        "p (m n) -> p m n", m=2)[:, :, poff::L][:, :, :B]
                                    nc.vector.tensor_tensor(
                                        out=ps[j][:].rearrange("p (m b) -> p m b", m=2),
                                        in0=ps[j][:].rearrange("p (m b) -> p m b", m=2),
                                        in1=pslice, op=mybir.AluOpType.add)
                            for j in range(3):
                                nc.scalar.activation(
                                    st["tmp"][:, j * 2 * B:(j + 1) * 2 * B],
                                    ps[j][:],
                                    mybir.ActivationFunctionType.Tanh,
                                    bias=0.0, scale=0.5)
                            nc.scalar.activation(
                                st["sig"][:], st["tmp"][:],
                                mybir.ActivationFunctionType.Copy,
                                bias=0.5, scale=0.5)
                            nc.scalar.activation(
                                st["tg"][:], ps[3][:],
                                mybir.ActivationFunctionType.Tanh)
                        # cell update (sig layout: [i | f | o] each 2B wide)
                        sig = st["sig"]
                        nc.vector.tensor_tensor(out=st["pr"][:],
                                                in0=sig[:, :2 * B],
                                                in1=st["tg"][:],
                                                op=mybir.AluOpType.mult)
                        nc.vector.tensor_tensor(out=st["c"][:],
                                                in0=sig[:, 2 * B:4 * B],
                                                in1=st["c"][:],
                                                op=mybir.AluOpType.mult)
                        nc.vector.tensor_tensor(out=st["c"][:],
                                                in0=st["c"][:],
                                                in1=st["pr"][:],
                                                op=mybir.AluOpType.add)
                        nc.scalar.activation(st["tc"][:], st["c"][:],
                                             mybir.ActivationFunctionType.Tanh)
                        hdst = st["hbf"] if t + 1 < WBF else st["h"]
                        nc.vector.tensor_tensor(out=hdst[:],
                                                in0=sig[:, 4 * B:6 * B],
                                                in1=st["tc"][:],
                                                op=mybir.AluOpType.mult)
                        # store h of real steps (and fwd warmup spill) to hfull
                        if d == 0:
                            if t >= W:
                                hoff = t - (W - WV)
                                for k in range(2):
                                    nc.vector.tensor_copy(
                                        hfull[0, k][:, hoff::L][:, :B],
                                        st["h"][:, k * B:(k + 1) * B])
                            elif t >= W - WV:
                                col = t - (W - WV)
                                hsrc = st["hbf"] if t + 1 < WBF else st["h"]
                                for k in range(2):
                                    nc.vector.tensor_copy(
                                        hfull[0, k][:, col:col + 1],
                                        hsrc[:, k * B:k * B + 1])
                        else:
                            if t >= W:
                                hoff = (L - 1) - (t - W)
                                for k in range(2):
                                    nc.vector.tensor_copy(
                                        hfull[1, k][:, hoff::L][:, :B],
                                        st["h"][:, k * B:(k + 1) * B])

            # ---- phase D+E: feats (PE) pipelined with viterbi scan (DVE) ----
            featsv = vpool.tile([P, VSTEP * T], f32)
            vhist = vpool.tile([P, VSTEP * T], f32)
            vvhist = vpool.tile([P, L * T], f32)
            schist = vpool.tile([P, L * T * T], f32)
            scscr = vpool.tile([P, T * T], f32)
            vmscr = vpool.tile([P, T], f32)
            trR3 = trR_sb[:].rearrange("p (a b) -> p a b", b=T)
            vprev = vinit_sb[:]
            sc_e = nc.enter_named_scope("viterbi", False)
            with tc.tile_pool(name="psD", bufs=1, space="PSUM") as psD:
                for t in range(VSTEP):
                    fv = psD.tile([P, T], f32, space="PSUM", tag="fvt",
                                  bufs=4, name="fvt")
                    for k4 in range(4):
                        d, k = divmod(k4, 2)
                        lhs = hfull[d, k][:, t::L][:, :P]
                        nc.tensor.matmul(fv[:], lhs, wtag_sb[k4][:],
                                         start=(k4 == 0), stop=(k4 == 3))
                    nc.vector.tensor_tensor(
                        out=featsv[:, t * T:(t + 1) * T], in0=fv[:],
                        in1=btag_sb[:, t * T:(t + 1) * T],
                        op=mybir.AluOpType.add)
                for t in range(VSTEP):
                    real = t >= WV
                    sc_ap = (schist[:, (t - WV) * T * T:(t - WV + 1) * T * T]
                             if real else scscr[:])
                    sc3 = sc_ap.rearrange("p (a b) -> p a b", b=T)
                    nc.vector.tensor_tensor(out=sc3, in0=bc_mid(vprev, T),
                                            in1=trR3, op=mybir.AluOpType.add)
                    vv_ap = (vvhist[:, (t - WV) * T:(t - WV + 1) * T]
                             if real else vmscr[:])
                    nc.vector.tensor_reduce(out=vv_ap, in_=sc3,
                                            axis=mybir.AxisListType.X,
                                            op=mybir.AluOpType.max)
                    vdst = vhist[:, t * T:(t + 1) * T]
                    nc.vector.tensor_tensor(out=vdst, in0=vv_ap,
                                            in1=featsv[:, t * T:(t + 1) * T],
                                            op=mybir.AluOpType.add)
                    if t < WV:
                        mb = vmask_sb[:, t:t + 1]
                        mask_bc = bass.AP(mb.tensor, mb.offset,
                                          [mb.ap[0], [0, T]])
                        nc.vector.copy_predicated(vdst, mask_bc, vprev)
                    vprev = vdst
            nc.sync.dma_start(out=o_feats[:], in_=featsv[:])
            # ---- phase F: backpointers ----
            mask = vpool.tile([P, L * T * T], f32)
            sch3 = schist[:].rearrange("p (a b) -> p a b", b=T)
            vvb = bc_last(vvhist[:], T)
            nc.vector.tensor_tensor(out=mask[:].rearrange("p (a b) -> p a b", b=T),
                                    in0=sch3, in1=vvb,
                                    op=mybir.AluOpType.is_equal)
            nc.vector.tensor_tensor(out=mask[:], in0=mask[:], in1=iot_sb[:],
                                    op=mybir.AluOpType.mult)
            r96 = vpool.tile([P, L * T], f32)
            nc.vector.tensor_reduce(out=r96[:],
                                    in_=mask[:].rearrange("p (a b) -> p a b", b=T),
                                    axis=mybir.AxisListType.X,
                                    op=mybir.AluOpType.max)
            bp_sb = vpool.tile([P, L * T], f32)
            nc.scalar.activation(bp_sb[:], r96[:],
                                 mybir.ActivationFunctionType.Copy,
                                 bias=float(T), scale=-1.0)
            nc.sync.dma_start(out=o_bptr[:], in_=bp_sb[:])
            nc.sync.dma_start(out=o_vhist[:], in_=vhist[:])
            nc.leave_named_scope("viterbi", sc_e[0], False)
    nc.compile()
    return nc


def _prep_static(emb_table, w_ih_f, w_hh_f, b_f, w_ih_b, w_hh_b, b_b,
                 w_tag, b_tag, transitions):
    """Host-side weight reordering/padding shared by all cores."""
    P = 128
    perm = np.r_[0:256, 256:512, 768:1024, 512:768]  # [i, f, o, g]
    out = {}
    out["emb"] = np.ascontiguousarray(emb_table.astype(np.float32))

    wihT = np.zeros((2, 3, P, G4), np.float32)
    whhT = np.zeros((2, 2, P, G4), np.float32)
    biasr = np.zeros((2, 8, P, 1), np.float32)
    for d, (wi, wh, bb_) in enumerate(((w_ih_f, w_hh_f, b_f),
                                       (w_ih_b, w_hh_b, b_b))):
        wiT = wi[perm].T.astype(np.float32)          # [300, 1024]
        wiTp = np.zeros((384, G4), np.float32)
        wiTp[:E] = wiT
        for k in range(3):
            wihT[d, k] = wiTp[128 * k:128 * (k + 1)]
        whT = wh[perm].T.astype(np.float32)          # [256, 1024]
        for k in range(2):
            whhT[d, k] = whT[128 * k:128 * (k + 1)]
        biasr[d, :, :, 0] = bb_[perm].astype(np.float32).reshape(8, P)
    out["wihT"], out["whhT"], out["biasr"] = wihT, whhT, biasr

    wtT = w_tag.T.astype(np.float32)                 # [512, 24]
    out["wtagT"] = wtT.reshape(4, P, T).copy()

    out["btag_rep"] = np.tile(b_tag.astype(np.float32)[None, :],
                              (P, VSTEP)).reshape(P, VSTEP * T).copy()
    out["transR"] = np.tile(transitions.astype(np.float32).reshape(1, T * T),
                            (P, 1)).copy()
    iot = np.tile((float(T) - np.arange(T, dtype=np.float32))[None, None, :],
                  (P, L * T, 1)).reshape(P, L * T * T)
    out["iot"] = np.ascontiguousarray(iot)
    vinit = np.full(T, NEG, np.float32)
    vinit[START] = 0.0
    out["vinit_rep"] = np.tile(vinit[None, :], (P, 1)).copy()
    return out


def kernel(sentence, emb_table, w_ih_f, w_hh_f, b_f, w_ih_b, w_hh_b, b_b,
           w_tag, b_tag, transitions):
    sentence = np.asarray(sentence)
    sent = sentence.astype(np.int64)
    trans = np.asarray(transitions, np.float32)

    if "nc" not in _PROG_CACHE:
        _PROG_CACHE["nc"] = _build_program()
    nc = _PROG_CACHE["nc"]

    static = _prep_static(np.asarray(emb_table), np.asarray(w_ih_f),
                          np.asarray(w_hh_f), np.asarray(b_f),
                          np.asarray(w_ih_b), np.asarray(w_hh_b),
                          np.asarray(b_b), np.asarray(w_tag),
                          np.asarray(b_tag), trans)

    in_maps = []
    for c in range(NCORES):
        m = dict(static)
        pos = np.arange(NPOS, dtype=np.int64) + (SPAN * c - XOFF)
        tokc = np.where((pos >= 0) & (pos < S), sent[np.clip(pos, 0, S - 1)],
                        OOB).astype(np.int32)
        m["tok"] = tokc
        vm = np.zeros((128, WV), np.int32)
        if c == 0:
            for b in range(WV // L):
                vm[b, :WV - L * b] = 1
        m["vmask"] = vm
        in_maps.append(m)

    trace = bool(os.environ.get("BASS_TRACE_KERNEL"))
    if trace:
        import ntff_shim  # noqa: F401
    res = run_bass_kernel_spmd(nc, in_maps, list(range(NCORES)), trace=trace)
    _PROG_CACHE["last_res"] = res

    # host postprocessing: backtrace + path score
    bp = np.zeros((S, T), np.int32)
    feats = np.zeros((S, T), np.float32)
    for c in range(NCORES):
        r = res.results[c]
        bpc = r["bptrs"].reshape(128, L, T)      # [chunk, t, to]
        fvc = r["featsv"].reshape(128, VSTEP, T)[:, WV:, :]
        bp[SPAN * c:SPAN * (c + 1)] = bpc.reshape(SPAN, T)
        feats[SPAN * c:SPAN * (c + 1)] = fvc.reshape(SPAN, T)

    v_end = res.results[NCORES - 1]["vhist"].reshape(128, VSTEP, T)[-1, -1]
    term = (v_end + trans[STOP]).astype(np.float32)
    best = int(np.argmax(term))
    path = np.zeros(S, np.int32)
    tag = best
    for t in range(S - 1, -1, -1):
        path[t] = tag
        tag = bp[t, tag]

    sc = np.float32(0.0)
    prev = START
    for t in range(S):
        sc = np.float32(np.float32(sc + trans[path[t], prev]) + feats[t, path[t]])
        prev = path[t]
    sc = np.float32(sc + trans[STOP, path[-1]])
    return np.float32(sc), path.astype(np.int32)


# revision 16
# speedup vs baseline: 1.0936x; 1.0936x over previous
"""BiLSTM-CRF Viterbi decode on 8 Trainium2 NeuronCores (Bass/Tile).

Strategy (self-contained, shapes hardcoded):
  - 8 cores, core c owns sentence positions [512c, 512(c+1)).
  - Embedding table replicated; each core indirect-DMA-gathers the 640
    token rows covering [512c-64, 512c+576) (out-of-range -> zero rows).
  - pre = x @ w_ih.T + b precomputed on PE for both directions.
  - LSTM recurrence: chunked with zero-init warmup. Per core two
    interleaved streams: forward (128 chunks x 4 tokens, warmup 48) and
    backward (136 chunks covering [512c-32, 512(c+1)), warmup 48).
    Gates are computed on PE (fp32), sigmoid via 0.5*tanh(x/2)+0.5 (the
    Tanh LUT is ~1 ulp), cell update on DVE. Warmup makes each chunk's
    state agree with the sequential scan to ~1 ulp (contraction of the
    LSTM map); boundary cores are exact via zero-x padding.
  - feats = h @ w_tag.T + b_tag computed directly in [chunk, (t, tag)]
    layout (144 small matmuls with column-strided stationary operands).
  - Viterbi: 128 chunks x 4 positions per core, warmup 32 from vinit
    (max-plus coalescence); core-0 short chunks are made exact with
    per-step copy_predicated restores. Backpointers recovered in one
    batched pass (is_equal + reversed-iota + max-reduce).
  - Host: backtrace over device backpointers, score re-accumulated in
    fp32 along the path (bitwise-faithful to the reference recursion).
"""
import os
import sys

for _p in ("/opt/trn_rl_repo", "/root/.axon_site/_ro/trn_rl_repo"):
    if os.path.isdir(_p) and _p not in sys.path:
        sys.path.append(_p)

import numpy as np

import concourse.bass as bass
import concourse.bacc as bacc
import concourse.mybir as mybir
import concourse.tile as tile
from concourse.bass_utils import run_bass_kernel_spmd
from concourse.masks import make_identity

f32 = mybir.dt.float32
i32 = mybir.dt.int32

# problem constants
S = 4096
V = 100000
E = 300
HH = 256
G4 = 1024
T = 24
START, STOP = 22, 23
NEG = -10000.0

NCORES = 8
SPAN = S // NCORES          # 512 positions per core
L = 4                       # tokens per LSTM/viterbi chunk
W = 36                      # LSTM warmup steps
WBF = 20                    # of which: bf16 hh-matmul head
WV = 24                     # viterbi warmup steps
BF = SPAN // L              # 128 forward chunks
BB = (SPAN + WV) // L       # 136 backward chunks (cover 32-pos left spill)
NSTEP = L + W               # 52 LSTM steps per stream
VSTEP = L + WV              # 36 viterbi steps
NPOS = 640                  # gathered x / pre positions: [512c-64, 512c+576)
XOFF = 64                   # position p -> pre column p - 512c + XOFF
HFULL = SPAN + WV           # 544 h columns: [512c-32, 512(c+1))
OOB = 1 << 20

_PROG_CACHE = {}


def _build_program():
    nc = bacc.Bacc("TRN2", target_bir_lowering=False, debug=False,
                   num_devices=NCORES)
    P = 128

    def din(name, shape, dt=f32):
        return nc.dram_tensor(name, shape, dt, kind="ExternalInput").ap()

    def dout(name, shape, dt=f32):
        return nc.dram_tensor(name, shape, dt, kind="ExternalOutput").ap()

    emb = din("emb", [V, E])
    tok = din("tok", [NPOS], i32)
    wihT = din("wihT", [2, 3, P, G4])       # [dir, ktile, 128, 1024] (E padded to 384)
    whhT = din("whhT", [2, 2, P, G4])
    biasr = din("biasr", [P, 16])
    wtagT = din("wtagT", [4, P, T])
    btag_rep = din("btag_rep", [P, VSTEP * T])
    transR = din("transR", [P, T * T])
    vmask = din("vmask", [P, WV], i32)
    iot = din("iot", [P, L * T * T])
    vinit_rep = din("vinit_rep", [P, T])

    o_bptr = dout("bptrs", [P, L * T])
    o_vhist = dout("vhist", [P, VSTEP * T])
    o_feats = dout("featsv", [P, VSTEP * T])

    def bc_mid(ap2, n):
        return bass.AP(ap2.tensor, ap2.offset, [ap2.ap[0], [0, n], ap2.ap[1]])

    def bc_last(ap2, n):
        return bass.AP(ap2.tensor, ap2.offset, [ap2.ap[0], ap2.ap[1], [0, n]])

    with tile.TileContext(nc) as tc:
        with (
            tc.tile_pool(name="const", bufs=1) as cpool,
            tc.tile_pool(name="state", bufs=1) as spool,
            tc.tile_pool(name="vit", bufs=1) as vpool,
        ):
            # ---- constants into SBUF ----
            ident = cpool.tile([P, P], f32)
            make_identity(nc, ident[:])

            # persistent LSTM state / outputs
            hfull = {}
            for d in range(2):
                for k in range(2):
                    t_ = spool.tile([P, HFULL], f32, tag=f"hfull{d}{k}", name=f"hfull{d}{k}")
                    hfull[d, k] = t_
            pre_f = spool.tile([P, 8 * NPOS], f32, tag="pref", name="pref")
            pre_pair = {}
            for j in range(4):
                t_ = spool.tile([P, 2 * NPOS], f32, tag=f"pre1{j}", name=f"pre1{j}")
                pre_pair[1, j] = t_

            # ---- phase A: gather x, build x^T ----
            with (
                tc.tile_pool(name="phA", bufs=1) as apool,
                tc.tile_pool(name="psA", bufs=2, space="PSUM") as psA,
                nc.named_scope("gather_pre"),
            ):
                tok_sb = apool.tile([P, 5], i32)
                nc.sync.dma_start(out=tok_sb[:],
                                  in_=tok.rearrange("(j p) -> p j", p=P))
                xg = []
                for j in range(5):
                    xt = apool.tile([P, E], f32, tag=f"x{j}", name=f"x{j}")
                    nc.vector.memset(xt[:], 0.0)
                    nc.gpsimd.indirect_dma_start(
                        out=xt[:], out_offset=None, in_=emb[:, :],
                        in_offset=bass.IndirectOffsetOnAxis(
                            ap=tok_sb[:, j:j + 1], axis=0),
                        bounds_check=V - 1, oob_is_err=False)
                    xg.append(xt)
                xT = []
                for k in range(3):
                    t_ = apool.tile([P, NPOS], f32, tag=f"xT{k}", name=f"xT{k}")
                    if k == 2:
                        nc.vector.memset(t_[:], 0.0)
                    xT.append(t_)
                for j in range(5):
                    for k in range(3):
                        ecols = min(128, E - 128 * k)
                        tp = psA.tile([P, P], f32, space="PSUM", tag="tp", name="tp")
                        nc.tensor.transpose(tp[:ecols, :],
                                            xg[j][:, 128 * k:128 * k + ecols],
                                            ident[:])
                        nc.vector.tensor_copy(
                            xT[k][:ecols, j * P:(j + 1) * P], tp[:ecols, :])

                # ---- phase B: pre = x @ w_ih^T + b ----
                bias_sb = cpool.tile([P, 16], f32)
                nc.sync.dma_start(out=bias_sb[:], in_=biasr[:])
                wih_sb = {}
                for d in range(2):
                    for k in range(3):
                        t_ = apool.tile([P, G4], f32, tag=f"wih{d}{k}", name=f"wih{d}{k}")
                        nc.sync.dma_start(out=t_[:], in_=wihT[d, k])
                        wih_sb[d, k] = t_
                with tc.tile_pool(name="psB", bufs=4, space="PSUM") as psB:
                    for d in range(2):
                        for m in range(8):
                            for ncol, c0 in ((512, 0), (128, 512)):
                                pm = psB.tile([P, 512], f32, space="PSUM",
                                              tag="pm", name="pm")
                                for k in range(3):
                                    nc.tensor.matmul(
                                        pm[:, :ncol],
                                        wih_sb[d, k][:, 128 * m:128 * (m + 1)],
                                        xT[k][:, c0:c0 + ncol],
                                        start=(k == 0), stop=(k == 2))
                                if d == 0:
                                    dst = pre_f[:, m * NPOS + c0:
                                                m * NPOS + c0 + ncol]
                                else:
                                    j, half = divmod(m, 2)
                                    dst = pre_pair[1, j][:, half * NPOS + c0:
                                                         half * NPOS + c0 + ncol]
                                nc.vector.tensor_scalar_add(
                                    dst, pm[:, :ncol],
                                    bias_sb[:, d * 8 + m:d * 8 + m + 1])

            whh_sb = {}
            for d in range(2):
                for k in range(2):
                    t_ = cpool.tile([P, G4], f32, tag=f"whh{d}{k}", name=f"whh{d}{k}")
                    nc.sync.dma_start(out=t_[:], in_=whhT[d, k])
                    whh_sb[d, k] = t_

            # ---- phase C: LSTM streams ----
            streams = [
                dict(d=0, B=BF),   # forward: single [128, 8B] psum (2 banks)
                dict(d=1, B=BB),   # backward: 4 psum tensors [128, 2B]
            ]
            with tc.tile_pool(name="psC", bufs=1, space="PSUM") as psC, \
                    nc.named_scope("lstm"):
                for st in streams:
                    B = st["B"]
                    d = st["d"]
                    st["h"] = spool.tile([P, 2 * B], f32, tag=f"h{d}", name=f"hcur{d}")
                    st["c"] = spool.tile([P, 2 * B], f32, tag=f"c{d}", name=f"ccur{d}")
                    st["tmp"] = spool.tile([P, 6 * B], f32, tag=f"tm{d}", name=f"tmp{d}")
                    st["sig"] = spool.tile([P, 6 * B], f32, tag=f"sg{d}", name=f"sig{d}")
                    st["tg"] = spool.tile([P, 2 * B], f32, tag=f"tg{d}", name=f"tgg{d}")
                    st["tc"] = spool.tile([P, 2 * B], f32, tag=f"tc{d}", name=f"tcc{d}")
                    st["pr"] = spool.tile([P, 2 * B], f32, tag=f"pr{d}", name=f"prd{d}")
                    st["hbf"] = spool.tile([P, 2 * B], mybir.dt.bfloat16,
                                           tag=f"hb{d}", name=f"hbf{d}")
                    nc.vector.memset(st["h"][:], 0.0)
                    nc.vector.memset(st["c"][:], 0.0)
                    nc.vector.memset(st["hbf"][:], 0.0)
                whh_bf = {}
                for d in range(2):
                    for k in range(2):
                        t_ = spool.tile([P, G4], mybir.dt.bfloat16,
                                        tag=f"whb{d}{k}", name=f"whb{d}{k}")
                        nc.vector.tensor_copy(t_[:], whh_sb[d, k][:])
                        whh_bf[d, k] = t_
                ident_bf = spool.tile([P, P], mybir.dt.bfloat16, name="identbf")
                nc.vector.tensor_copy(ident_bf[:], ident[:])
                pre_f_bf = spool.tile([P, 8 * NPOS], mybir.dt.bfloat16,
                                      tag="prefbf", name="prefbf")
                nc.vector.tensor_copy(pre_f_bf[:], pre_f[:])
                pre_bf1 = {}
                for j in range(4):
                    t_ = spool.tile([P, 2 * NPOS], mybir.dt.bfloat16,
                                    tag=f"preb1{j}", name=f"preb1{j}")
                    nc.vector.tensor_copy(t_[:], pre_pair[1, j][:])
                    pre_bf1[j] = t_

                for t in range(NSTEP):
                    for st in streams:
                        d, B = st["d"], st["B"]
                        if d == 0:
                            poff = (XOFF - W) + t
                            ps = [psC.tile([P, 2 * B], f32, space="PSUM",
                                           tag=f"psf{j}", name=f"psf{j}")
                                  for j in range(4)]
                            bf = t < WBF
                            wsel = whh_bf if bf else whh_sb
                            hsel = st["hbf"] if bf else st["h"]
                            for j in range(4):
                                for half in range(2):
                                    m = 2 * j + half
                                    if bf:
                                        pslice = pre_f_bf[:, m * NPOS + poff::L][:, :B]
                                        nc.tensor.matmul(
                                            ps[j][:, half * B:(half + 1) * B],
                                            ident_bf[:], pslice,
                                            start=True, stop=False)
                                    for k in range(2):
                                        nc.tensor.matmul(
                                            ps[j][:, half * B:(half + 1) * B],
                                            wsel[0, k][:, 128 * m:128 * (m + 1)],
                                            hsel[:, k * B:(k + 1) * B],
                                            start=(k == 0 and not bf),
                                            stop=(k == 1))
                            if not bf:
                                for j in range(4):
                                    pslice = pre_f[:].rearrange(
                                        "p (m n) -> p m n", m=8)[:, 2 * j:2 * j + 2,
                                                                poff::L][:, :, :B]
                                    nc.vector.tensor_tensor(
                                        out=ps[j][:].rearrange("p (m b) -> p m b", m=2),
                                        in0=ps[j][:].rearrange("p (m b) -> p m b", m=2),
                                        in1=pslice, op=mybir.AluOpType.add)
                            if bf:
                                for j in range(3):
                                    nc.scalar.activation(
                                        st["sig"][:, j * 2 * B:(j + 1) * 2 * B],
                                        ps[j][:],
                                        mybir.ActivationFunctionType.Sigmoid)
                            else:
                                for j in range(3):
                                    nc.scalar.activation(
                                        st["tmp"][:, j * 2 * B:(j + 1) * 2 * B],
                                        ps[j][:],
                                        mybir.ActivationFunctionType.Tanh,
                                        bias=0.0, scale=0.5)
                                nc.scalar.activation(
                                    st["sig"][:], st["tmp"][:],
                                    mybir.ActivationFunctionType.Copy,
                                    bias=0.5, scale=0.5)
                            nc.scalar.activation(
                                st["tg"][:], ps[3][:],
                                mybir.ActivationFunctionType.Tanh)
                        else:
                            poff = (XOFF - WV) + (L - 1) + W - t
                            ps = [psC.tile([P, 2 * B], f32, space="PSUM",
                                           tag=f"psb{j}", name=f"psb{j}")
                                  for j in range(4)]
                            bf = t < WBF
                            wsel = whh_bf if bf else whh_sb
                            hsel = st["hbf"] if bf else st["h"]
                            for j in range(4):
                                for half in range(2):
                                    m = 2 * j + half
                                    if bf:
                                        pslice = pre_bf1[j][:, half * NPOS + poff::L][:, :B]
                                        nc.tensor.matmul(
                                            ps[j][:, half * B:(half + 1) * B],
                                            ident_bf[:], pslice,
                                            start=True, stop=False)
                                    for k in range(2):
                                        nc.tensor.matmul(
                                            ps[j][:, half * B:(half + 1) * B],
                                            wsel[1, k][:, 128 * m:128 * (m + 1)],
                                            hsel[:, k * B:(k + 1) * B],
                                            start=(k == 0 and not bf),
                                            stop=(k == 1))
                            if not bf:
                                for j in range(4):
                                    pslice = pre_pair[1, j][:].rearrange(
                                        "p (m n) -> p m n", m=2)[:, :, poff::L][:, :, :B]
                                    nc.vector.tensor_tensor(
                                        out=ps[j][:].rearrange("p (m b) -> p m b", m=2),
                                        in0=ps[j][:].rearrange("p (m b) -> p m b", m=2),
                                        in1=pslice, op=mybir.AluOpType.add)
                            if bf:
                                for j in range(3):
                                    nc.scalar.activation(
                                        st["sig"][:, j * 2 * B:(j + 1) * 2 * B],
                                        ps[j][:],
                                        mybir.ActivationFunctionType.Sigmoid)
                            else:
                                for j in range(3):
                                    nc.scalar.activation(
                                        st["tmp"][:, j * 2 * B:(j + 1) * 2 * B],
                                        ps[j][:],
                                        mybir.ActivationFunctionType.Tanh,
                                        bias=0.0, scale=0.5)
                                nc.scalar.activation(
                                    st["sig"][:], st["tmp"][:],
                                    mybir.ActivationFunctionType.Copy,
                                    bias=0.5, scale=0.5)
                            nc.scalar.activation(
                                st["tg"][:], ps[3][:],
                                mybir.ActivationFunctionType.Tanh)
                        # cell update (sig layout: [i | f | o] each 2B wide)
                        sig = st["sig"]
                        nc.vector.tensor_tensor(out=st["pr"][:],
                                                in0=sig[:, :2 * B],
                                                in1=st["tg"][:],
                                                op=mybir.AluOpType.mult)
                        nc.vector.tensor_tensor(out=st["c"][:],
                                                in0=sig[:, 2 * B:4 * B],
                                                in1=st["c"][:],
                                                op=mybir.AluOpType.mult)
                        nc.vector.tensor_tensor(out=st["c"][:],
                                                in0=st["c"][:],
                                                in1=st["pr"][:],
                                                op=mybir.AluOpType.add)
                        nc.scalar.activation(st["tc"][:], st["c"][:],
                                             mybir.ActivationFunctionType.Tanh)
                        hdst = st["hbf"] if t + 1 < WBF else st["h"]
                        nc.vector.tensor_tensor(out=hdst[:],
                                                in0=sig[:, 4 * B:6 * B],
                                                in1=st["tc"][:],
                                                op=mybir.AluOpType.mult)
                        # store h of real steps (and fwd warmup spill) to hfull
                        if d == 0:
                            if t >= W:
                                hoff = t - (W - WV)
                                for k in range(2):
                                    nc.vector.tensor_copy(
                                        hfull[0, k][:, hoff::L][:, :B],
                                        st["h"][:, k * B:(k + 1) * B])
                            elif t >= W - WV:
                                col = t - (W - WV)
                                hsrc = st["hbf"] if t + 1 < WBF else st["h"]
                                for k in range(2):
                                    nc.vector.tensor_copy(
                                        hfull[0, k][:, col:col + 1],
                                        hsrc[:, k * B:k * B + 1])
                        else:
                            if t >= W:
                                hoff = (L - 1) - (t - W)
                                for k in range(2):
                                    nc.vector.tensor_copy(
                                        hfull[1, k][:, hoff::L][:, :B],
                                        st["h"][:, k * B:(k + 1) * B])

            wtag_sb = []
            for k in range(4):
                t_ = cpool.tile([P, T], f32, tag=f"wtag{k}", name=f"wtag{k}")
                nc.sync.dma_start(out=t_[:], in_=wtagT[k])
                wtag_sb.append(t_)
            btag_sb = vpool.tile([P, VSTEP * T], f32)
            nc.sync.dma_start(out=btag_sb[:], in_=btag_rep[:])
            trR_sb = vpool.tile([P, T * T], f32)
            nc.sync.dma_start(out=trR_sb[:], in_=transR[:])
            vmask_sb = vpool.tile([P, WV], i32)
            nc.sync.dma_start(out=vmask_sb[:], in_=vmask[:])
            iot_sb = vpool.tile([P, L * T * T], f32)
            nc.sync.dma_start(out=iot_sb[:], in_=iot[:])
            vinit_sb = vpool.tile([P, T], f32)
            nc.sync.dma_start(out=vinit_sb[:], in_=vinit_rep[:])

            # ---- phase D+E: feats (PE) pipelined with viterbi scan (DVE) ----
            featsv = vpool.tile([P, VSTEP * T], f32)
            vhist = vpool.tile([P, VSTEP * T], f32)
            vvhist = vpool.tile([P, L * T], f32)
            schist = vpool.tile([P, L * T * T], f32)
            scscr = vpool.tile([P, T * T], f32)
            vmscr = vpool.tile([P, T], f32)
            trR3 = trR_sb[:].rearrange("p (a b) -> p a b", b=T)
            vprev = vinit_sb[:]
            sc_e = nc.enter_named_scope("viterbi", False)
            with tc.tile_pool(name="psD", bufs=1, space="PSUM") as psD:
                for t in range(VSTEP):
                    fv = psD.tile([P, T], f32, space="PSUM", tag="fvt",
                                  bufs=4, name="fvt")
                    for k4 in range(4):
                        d, k = divmod(k4, 2)
                        lhs = hfull[d, k][:, t::L][:, :P]
                        nc.tensor.matmul(fv[:], lhs, wtag_sb[k4][:],
                                         start=(k4 == 0), stop=(k4 == 3))
                    nc.vector.tensor_tensor(
                        out=featsv[:, t * T:(t + 1) * T], in0=fv[:],
                        in1=btag_sb[:, t * T:(t + 1) * T],
                        op=mybir.AluOpType.add)
                    real = t >= WV
                    sc_ap = (schist[:, (t - WV) * T * T:(t - WV + 1) * T * T]
                             if real else scscr[:])
                    sc3 = sc_ap.rearrange("p (a b) -> p a b", b=T)
                    nc.vector.tensor_tensor(out=sc3, in0=bc_mid(vprev, T),
                                            in1=trR3, op=mybir.AluOpType.add)
                    vv_ap = (vvhist[:, (t - WV) * T:(t - WV + 1) * T]
                             if real else vmscr[:])
                    nc.vector.tensor_reduce(out=vv_ap, in_=sc3,
                                            axis=mybir.AxisListType.X,
                                            op=mybir.AluOpType.max)
                    vdst = vhist[:, t * T:(t + 1) * T]
                    nc.vector.tensor_tensor(out=vdst, in0=vv_ap,
                                            in1=featsv[:, t * T:(t + 1) * T],
                                            op=mybir.AluOpType.add)
                    if t < WV:
                        mb = vmask_sb[:, t:t + 1]
                        mask_bc = bass.AP(mb.tensor, mb.offset,
                                          [mb.ap[0], [0, T]])
                        nc.vector.copy_predicated(vdst, mask_bc, vprev)
                    vprev = vdst
            nc.sync.dma_start(out=o_feats[:], in_=featsv[:])
            # ---- phase F: backpointers ----
            mask = vpool.tile([P, L * T * T], f32)
            sch3 = schist[:].rearrange("p (a b) -> p a b", b=T)
            vvb = bc_last(vvhist[:], T)
            nc.vector.tensor_tensor(out=mask[:].rearrange("p (a b) -> p a b", b=T),
                                    in0=sch3, in1=vvb,
                                    op=mybir.AluOpType.is_equal)
            nc.vector.tensor_tensor(out=mask[:], in0=mask[:], in1=iot_sb[:],
                                    op=mybir.AluOpType.mult)
            r96 = vpool.tile([P, L * T], f32)
            nc.vector.tensor_reduce(out=r96[:],
                                    in_=mask[:].rearrange("p (a b) -> p a b", b=T),
                                    axis=mybir.AxisListType.X,
                                    op=mybir.AluOpType.max)
            bp_sb = vpool.tile([P, L * T], f32)
            nc.scalar.activation(bp_sb[:], r96[:],
                                 mybir.ActivationFunctionType.Copy,
                                 bias=float(T), scale=-1.0)
            nc.sync.dma_start(out=o_bptr[:], in_=bp_sb[:])
            nc.sync.dma_start(out=o_vhist[:], in_=vhist[:])
            nc.leave_named_scope("viterbi", sc_e[0], False)
    nc.compile()
    return nc


def _prep_static(emb_table, w_ih_f, w_hh_f, b_f, w_ih_b, w_hh_b, b_b,
                 w_tag, b_tag, transitions):
    """Host-side weight reordering/padding shared by all cores."""
    P = 128
    perm = np.r_[0:256, 256:512, 768:1024, 512:768]  # [i, f, o, g]
    out = {}
    out["emb"] = np.ascontiguousarray(emb_table.astype(np.float32))

    wihT = np.zeros((2, 3, P, G4), np.float32)
    whhT = np.zeros((2, 2, P, G4), np.float32)
    biasr = np.zeros((P, 16), np.float32)
    for d, (wi, wh, bb_) in enumerate(((w_ih_f, w_hh_f, b_f),
                                       (w_ih_b, w_hh_b, b_b))):
        wiT = wi[perm].T.astype(np.float32)          # [300, 1024]
        wiTp = np.zeros((384, G4), np.float32)
        wiTp[:E] = wiT
        for k in range(3):
            wihT[d, k] = wiTp[128 * k:128 * (k + 1)]
        whT = wh[perm].T.astype(np.float32)          # [256, 1024]
        for k in range(2):
            whhT[d, k] = whT[128 * k:128 * (k + 1)]
        biasr[:, d * 8:(d + 1) * 8] = bb_[perm].astype(np.float32).reshape(8, P).T
    out["wihT"], out["whhT"], out["biasr"] = wihT, whhT, biasr

    wtT = w_tag.T.astype(np.float32)                 # [512, 24]
    out["wtagT"] = wtT.reshape(4, P, T).copy()

    out["btag_rep"] = np.tile(b_tag.astype(np.float32)[None, :],
                              (P, VSTEP)).reshape(P, VSTEP * T).copy()
    out["transR"] = np.tile(transitions.astype(np.float32).reshape(1, T * T),
                            (P, 1)).copy()
    iot = np.tile((float(T) - np.arange(T, dtype=np.float32))[None, None, :],
                  (P, L * T, 1)).reshape(P, L * T * T)
    out["iot"] = np.ascontiguousarray(iot)
    vinit = np.full(T, NEG, np.float32)
    vinit[START] = 0.0
    out["vinit_rep"] = np.tile(vinit[None, :], (P, 1)).copy()
    return out


def kernel(sentence, emb_table, w_ih_f, w_hh_f, b_f, w_ih_b, w_hh_b, b_b,
           w_tag, b_tag, transitions):
    sentence = np.asarray(sentence)
    sent = sentence.astype(np.int64)
    trans = np.asarray(transitions, np.float32)

    if "nc" not in _PROG_CACHE:
        _PROG_CACHE["nc"] = _build_program()
    nc = _PROG_CACHE["nc"]

    static = _prep_static(np.asarray(emb_table), np.asarray(w_ih_f),
                          np.asarray(w_hh_f), np.asarray(b_f),
                          np.asarray(w_ih_b), np.asarray(w_hh_b),
                          np.asarray(b_b), np.asarray(w_tag),
                          np.asarray(b_tag), trans)

    in_maps = []
    for c in range(NCORES):
        m = dict(static)
        pos = np.arange(NPOS, dtype=np.int64) + (SPAN * c - XOFF)
        tokc = np.where((pos >= 0) & (pos < S), sent[np.clip(pos, 0, S - 1)],
                        OOB).astype(np.int32)
        m["tok"] = tokc
        vm = np.zeros((128, WV), np.int32)
        if c == 0:
            for b in range(WV // L):
                vm[b, :WV - L * b] = 1
        m["vmask"] = vm
        in_maps.append(m)

    trace = bool(os.environ.get("BASS_TRACE_KERNEL"))
    if trace:
        import ntff_shim  # noqa: F401
    res = run_bass_kernel_spmd(nc, in_maps, list(range(NCORES)), trace=trace)
    _PROG_CACHE["last_res"] = res

    # host postprocessing: backtrace + path score
    bp = np.zeros((S, T), np.int32)
    feats = np.zeros((S, T), np.float32)
    for c in range(NCORES):
        r = res.results[c]
        bpc = r["bptrs"].reshape(128, L, T)      # [chunk, t, to]
        fvc = r["featsv"].reshape(128, VSTEP, T)[:, WV:, :]
        bp[SPAN * c:SPAN * (c + 1)] = bpc.reshape(SPAN, T)
        feats[SPAN * c:SPAN * (c + 1)] = fvc.reshape(SPAN, T)

    v_end = res.results[NCORES - 1]["vhist"].reshape(128, VSTEP, T)[-1, -1]
    term = (v_end + trans[STOP]).astype(np.float32)
    best = int(np.argmax(term))
    path = np.zeros(S, np.int32)
    tag = best
    for t in range(S - 1, -1, -1):
        path[t] = tag
        tag = bp[t, tag]

    sc = np.float32(0.0)
    prev = START
    for t in range(S):
        sc = np.float32(np.float32(sc + trans[path[t], prev]) + feats[t, path[t]])
        prev = path[t]
    sc = np.float32(sc + trans[STOP, path[-1]])
    return np.float32(sc), path.astype(np.int32)


# revision 18
# speedup vs baseline: 1.1057x; 1.0111x over previous
"""BiLSTM-CRF Viterbi decode on 8 Trainium2 NeuronCores (Bass/Tile).

Strategy (self-contained, shapes hardcoded):
  - 8 cores, core c owns sentence positions [512c, 512(c+1)).
  - Embedding table replicated; each core indirect-DMA-gathers the 640
    token rows covering [512c-64, 512c+576) (out-of-range -> zero rows).
  - pre = x @ w_ih.T + b precomputed on PE for both directions.
  - LSTM recurrence: chunked with zero-init warmup. Per core two
    interleaved streams: forward (128 chunks x 4 tokens, warmup 48) and
    backward (136 chunks covering [512c-32, 512(c+1)), warmup 48).
    Gates are computed on PE (fp32), sigmoid via 0.5*tanh(x/2)+0.5 (the
    Tanh LUT is ~1 ulp), cell update on DVE. Warmup makes each chunk's
    state agree with the sequential scan to ~1 ulp (contraction of the
    LSTM map); boundary cores are exact via zero-x padding.
  - feats = h @ w_tag.T + b_tag computed directly in [chunk, (t, tag)]
    layout (144 small matmuls with column-strided stationary operands).
  - Viterbi: 128 chunks x 4 positions per core, warmup 32 from vinit
    (max-plus coalescence); core-0 short chunks are made exact with
    per-step copy_predicated restores. Backpointers recovered in one
    batched pass (is_equal + reversed-iota + max-reduce).
  - Host: backtrace over device backpointers, score re-accumulated in
    fp32 along the path (bitwise-faithful to the reference recursion).
"""
import os
import sys

for _p in ("/opt/trn_rl_repo", "/root/.axon_site/_ro/trn_rl_repo"):
    if os.path.isdir(_p) and _p not in sys.path:
        sys.path.append(_p)

import numpy as np

import concourse.bass as bass
import concourse.bacc as bacc
import concourse.mybir as mybir
import concourse.tile as tile
from concourse.bass_utils import run_bass_kernel_spmd
from concourse.masks import make_identity

f32 = mybir.dt.float32
i32 = mybir.dt.int32

# problem constants
S = 4096
V = 100000
E = 300
HH = 256
G4 = 1024
T = 24
START, STOP = 22, 23
NEG = -10000.0

NCORES = 8
SPAN = S // NCORES          # 512 positions per core
L = 4                       # tokens per LSTM/viterbi chunk
W = 34                      # LSTM warmup steps
WBF = 18                    # of which: bf16 hh-matmul head
WV = 16                     # viterbi warmup steps
BF = SPAN // L              # 128 forward chunks
BB = (SPAN + WV) // L       # 136 backward chunks (cover 32-pos left spill)
NSTEP = L + W               # 52 LSTM steps per stream
VSTEP = L + WV              # 36 viterbi steps
NPOS = 640                  # gathered x / pre positions: [512c-64, 512c+576)
XOFF = 64                   # position p -> pre column p - 512c + XOFF
HFULL = SPAN + WV           # 544 h columns: [512c-32, 512(c+1))
OOB = 1 << 20

_PROG_CACHE = {}


def _build_program():
    nc = bacc.Bacc("TRN2", target_bir_lowering=False, debug=False,
                   num_devices=NCORES)
    P = 128
    bf16 = mybir.dt.bfloat16

    def din(name, shape, dt=f32):
        return nc.dram_tensor(name, shape, dt, kind="ExternalInput").ap()

    def dout(name, shape, dt=f32):
        return nc.dram_tensor(name, shape, dt, kind="ExternalOutput").ap()

    emb = din("emb", [V, E])
    tok = din("tok", [NPOS], i32)
    wihT = din("wihT", [2, 3, P, G4])
    whhT = din("whhT", [2, 2, P, G4])
    biasr = din("biasr", [P, 16])
    wtagT = din("wtagT", [4, P, T])
    btag_rep = din("btag_rep", [P, VSTEP * T])
    transR = din("transR", [P, T * T])
    vmask = din("vmask", [P, WV], i32)
    iot = din("iot", [P, L * T * T])
    vinit_rep = din("vinit_rep", [P, T])

    o_bptr = dout("bptrs", [P, L * T])
    o_vhist = dout("vhist", [P, VSTEP * T])
    o_feats = dout("featsv", [P, VSTEP * T])

    def bc_mid(ap2, n):
        return bass.AP(ap2.tensor, ap2.offset, [ap2.ap[0], [0, n], ap2.ap[1]])

    def bc_last(ap2, n):
        return bass.AP(ap2.tensor, ap2.offset, [ap2.ap[0], ap2.ap[1], [0, n]])

    with tile.TileContext(nc) as tc:
        with (
            tc.tile_pool(name="const", bufs=1) as cpool,
            tc.tile_pool(name="state", bufs=1) as spool,
        ):
            ident = cpool.tile([P, P], f32)
            make_identity(nc, ident[:])
            ident_bf = cpool.tile([P, P], bf16, name="identbf")
            nc.vector.tensor_copy(ident_bf[:], ident[:])

            hfull = {}
            for d in range(2):
                for k in range(2):
                    t_ = spool.tile([P, HFULL], f32, tag=f"hfull{d}{k}", name=f"hfull{d}{k}")
                    hfull[d, k] = t_
            pre_f = spool.tile([P, 8 * NPOS], f32, tag="pref", name="pref")
            pre_pair = {}
            for j in range(4):
                t_ = spool.tile([P, 2 * NPOS], f32, tag=f"pre1{j}", name=f"pre1{j}")
                pre_pair[1, j] = t_
            pre_f_bf = spool.tile([P, 8 * NPOS], bf16, tag="prefbf", name="prefbf")
            pre_bf1 = {}
            for j in range(4):
                t_ = spool.tile([P, 2 * NPOS], bf16, tag=f"preb1{j}", name=f"preb1{j}")
                pre_bf1[j] = t_

            with tc.tile_pool(name="wih", bufs=1) as wpool, \
                    tc.tile_pool(name="psC", bufs=1, space="PSUM") as psC:
                # ---- phase A: gather x, transpose, casts ----
                with tc.tile_pool(name="phA", bufs=1) as apool:
                    tok_sb = apool.tile([P, 5], i32)
                    nc.sync.dma_start(out=tok_sb[:],
                                      in_=tok.rearrange("(j p) -> p j", p=P))
                    xg = []
                    for j in range(5):
                        xt = apool.tile([P, E], f32, tag=f"x{j}", name=f"x{j}")
                        nc.vector.memset(xt[:], 0.0)
                        nc.gpsimd.indirect_dma_start(
                            out=xt[:], out_offset=None, in_=emb[:, :],
                            in_offset=bass.IndirectOffsetOnAxis(
                                ap=tok_sb[:, j:j + 1], axis=0),
                            bounds_check=V - 1, oob_is_err=False)
                        xg.append(xt)
                    xT = []
                    xT_bf = []
                    for k in range(3):
                        t_ = wpool.tile([P, NPOS], f32, tag=f"xT{k}", name=f"xT{k}")
                        if k == 2:
                            nc.vector.memset(t_[:], 0.0)
                        xT.append(t_)
                        tb_ = wpool.tile([P, NPOS], bf16, tag=f"xTb{k}", name=f"xTb{k}")
                        if k == 2:
                            nc.vector.memset(tb_[:], 0)
                        xT_bf.append(tb_)
                    for j in range(5):
                        for k in range(3):
                            ecols = min(128, E - 128 * k)
                            tp = psC.tile([P, P], f32, space="PSUM", tag="pm",
                                          bufs=2, name="tp")
                            nc.tensor.transpose(tp[:ecols, :],
                                                xg[j][:, 128 * k:128 * k + ecols],
                                                ident[:])
                            nc.vector.tensor_copy(
                                xT[k][:ecols, j * P:(j + 1) * P], tp[:ecols, :])
                            nc.vector.tensor_copy(
                                xT_bf[k][:ecols, j * P:(j + 1) * P], tp[:ecols, :])
                    bias_sb = cpool.tile([P, 16], f32)
                    nc.sync.dma_start(out=bias_sb[:], in_=biasr[:])
                    wih_sb = {}
                    wih_bf = {}
                    for d in range(2):
                        for k in range(3):
                            t_ = wpool.tile([P, G4], f32, tag=f"wih{d}{k}", name=f"wih{d}{k}")
                            nc.sync.dma_start(out=t_[:], in_=wihT[d, k])
                            wih_sb[d, k] = t_
                            tb_ = wpool.tile([P, G4], bf16, tag=f"wihb{d}{k}", name=f"wihb{d}{k}")
                            nc.vector.tensor_copy(tb_[:], t_[:])
                            wih_bf[d, k] = tb_

                # ---- phase B-bf16: bf16 pre for the warmup head ----
                def pre_mm(d, m, ncol, c0, use_bf):
                    pm = psC.tile([P, 512], f32, space="PSUM",
                                  tag="pm", bufs=2, name="pm")
                    wsrc = wih_bf if use_bf else wih_sb
                    xsrc = xT_bf if use_bf else xT
                    for k in range(3):
                        nc.tensor.matmul(
                            pm[:, :ncol],
                            wsrc[d, k][:, 128 * m:128 * (m + 1)],
                            xsrc[k][:, c0:c0 + ncol],
                            start=(k == 0), stop=(k == 2))
                    if use_bf:
                        if d == 0:
                            dst = pre_f_bf[:, m * NPOS + c0:m * NPOS + c0 + ncol]
                        else:
                            j, half = divmod(m, 2)
                            dst = pre_bf1[j][:, half * NPOS + c0:
                                             half * NPOS + c0 + ncol]
                    else:
                        if d == 0:
                            dst = pre_f[:, m * NPOS + c0:m * NPOS + c0 + ncol]
                        else:
                            j, half = divmod(m, 2)
                            dst = pre_pair[1, j][:, half * NPOS + c0:
                                                 half * NPOS + c0 + ncol]
                    nc.vector.tensor_scalar_add(
                        dst, pm[:, :ncol], bias_sb[:, d * 8 + m:d * 8 + m + 1])

                for d in range(2):
                    for m in range(8):
                        for ncol, c0 in ((512, 0), (128, 512)):
                            pre_mm(d, m, ncol, c0, True)
                fp32_pre_jobs = [(d, m, ncol, c0)
                                 for d in range(2) for m in range(8)
                                 for ncol, c0 in ((512, 0), (128, 512))]

                whh_sb = {}
                whh_bf = {}
                for d in range(2):
                    for k in range(2):
                        t_ = cpool.tile([P, G4], f32, tag=f"whh{d}{k}", name=f"whh{d}{k}")
                        nc.sync.dma_start(out=t_[:], in_=whhT[d, k])
                        whh_sb[d, k] = t_
                        tb_ = cpool.tile([P, G4], bf16, tag=f"whb{d}{k}", name=f"whb{d}{k}")
                        nc.vector.tensor_copy(tb_[:], t_[:])
                        whh_bf[d, k] = tb_

                # ---- phase C: LSTM streams ----
                streams = [dict(d=0, B=BF), dict(d=1, B=BB)]
                with nc.named_scope("lstm"):
                    for st in streams:
                        B = st["B"]
                        d = st["d"]
                        st["h"] = spool.tile([P, 2 * B], f32, tag=f"h{d}", name=f"hcur{d}")
                        st["c"] = spool.tile([P, 2 * B], f32, tag=f"c{d}", name=f"ccur{d}")
                        st["tmp"] = spool.tile([P, 6 * B], f32, tag=f"tm{d}", name=f"tmp{d}")
                        st["sig"] = spool.tile([P, 6 * B], f32, tag=f"sg{d}", name=f"sig{d}")
                        st["tg"] = spool.tile([P, 2 * B], f32, tag=f"tg{d}", name=f"tgg{d}")
                        st["tc"] = spool.tile([P, 2 * B], f32, tag=f"tc{d}", name=f"tcc{d}")
                        st["pr"] = spool.tile([P, 2 * B], f32, tag=f"pr{d}", name=f"prd{d}")
                        st["hbf"] = spool.tile([P, 2 * B], bf16, tag=f"hb{d}", name=f"hbf{d}")
                        nc.vector.memset(st["h"][:], 0.0)
                        nc.vector.memset(st["c"][:], 0.0)
                        nc.vector.memset(st["hbf"][:], 0)

                    for t in range(NSTEP):
                        for st in streams:
                            d, B = st["d"], st["B"]
                            bf = t < WBF
                            wsel = whh_bf if bf else whh_sb
                            hsel = st["hbf"] if bf else st["h"]
                            if d == 0:
                                poff = (XOFF - W) + t
                                ps = [psC.tile([P, 4 * B], f32, space="PSUM",
                                               tag=f"psf{jj}", name=f"psf{jj}")
                                      for jj in range(2)]
                                for m in range(8):
                                    half, mm = divmod(m, 4)
                                    dstp = ps[half][:, mm * B:(mm + 1) * B]
                                    if bf:
                                        pslice = pre_f_bf[:, m * NPOS + poff::L][:, :B]
                                        nc.tensor.matmul(dstp, ident_bf[:], pslice,
                                                         start=True, stop=False)
                                    for k in range(2):
                                        nc.tensor.matmul(
                                            dstp,
                                            wsel[0, k][:, 128 * m:128 * (m + 1)],
                                            hsel[:, k * B:(k + 1) * B],
                                            start=(k == 0 and not bf),
                                            stop=(k == 1))
                                if not bf:
                                    for half in range(2):
                                        pslice = pre_f[:].rearrange(
                                            "p (m n) -> p m n", m=8)[:, 4 * half:4 * half + 4,
                                                                     poff::L][:, :, :B]
                                        nc.vector.tensor_tensor(
                                            out=ps[half][:].rearrange("p (m b) -> p m b", m=4),
                                            in0=ps[half][:].rearrange("p (m b) -> p m b", m=4),
                                            in1=pslice, op=mybir.AluOpType.add)
                                if bf:
                                    nc.scalar.activation(
                                        st["sig"][:, :4 * B], ps[0][:],
                                        mybir.ActivationFunctionType.Sigmoid)
                                    nc.scalar.activation(
                                        st["sig"][:, 4 * B:6 * B], ps[1][:, :2 * B],
                                        mybir.ActivationFunctionType.Sigmoid)
                                else:
                                    nc.scalar.activation(
                                        st["tmp"][:, :4 * B], ps[0][:],
                                        mybir.ActivationFunctionType.Tanh,
                                        bias=0.0, scale=0.5)
                                    nc.scalar.activation(
                                        st["tmp"][:, 4 * B:6 * B], ps[1][:, :2 * B],
                                        mybir.ActivationFunctionType.Tanh,
                                        bias=0.0, scale=0.5)
                                    nc.scalar.activation(
                                        st["sig"][:], st["tmp"][:],
                                        mybir.ActivationFunctionType.Copy,
                                        bias=0.5, scale=0.5)
                                nc.scalar.activation(
                                    st["tg"][:], ps[1][:, 2 * B:4 * B],
                                    mybir.ActivationFunctionType.Tanh)
                            else:
                                poff = (XOFF - WV) + (L - 1) + W - t
                                ps = [psC.tile([P, 2 * B], f32, space="PSUM",
                                               tag=f"psb{jj}", name=f"psb{jj}")
                                      for jj in range(4)]
                                for j in range(4):
                                    for half in range(2):
                                        m = 2 * j + half
                                        dstp = ps[j][:, half * B:(half + 1) * B]
                                        if bf:
                                            pslice = pre_bf1[j][:, half * NPOS + poff::L][:, :B]
                                            nc.tensor.matmul(dstp, ident_bf[:], pslice,
                                                             start=True, stop=False)
                                        for k in range(2):
                                            nc.tensor.matmul(
                                                dstp,
                                                wsel[1, k][:, 128 * m:128 * (m + 1)],
                                                hsel[:, k * B:(k + 1) * B],
                                                start=(k == 0 and not bf),
                                                stop=(k == 1))
                                if not bf:
                                    for j in range(4):
                                        pslice = pre_pair[1, j][:].rearrange(
                                            "p (m n) -> p m n", m=2)[:, :, poff::L][:, :, :B]
                                        nc.vector.tensor_tensor(
                                            out=ps[j][:].rearrange("p (m b) -> p m b", m=2),
                                            in0=ps[j][:].rearrange("p (m b) -> p m b", m=2),
                                            in1=pslice, op=mybir.AluOpType.add)
                                if bf:
                                    for j in range(3):
                                        nc.scalar.activation(
                                            st["sig"][:, j * 2 * B:(j + 1) * 2 * B],
                                            ps[j][:],
                                            mybir.ActivationFunctionType.Sigmoid)
                                else:
                                    for j in range(3):
                                        nc.scalar.activation(
                                            st["tmp"][:, j * 2 * B:(j + 1) * 2 * B],
                                            ps[j][:],
                                            mybir.ActivationFunctionType.Tanh,
                                            bias=0.0, scale=0.5)
                                    nc.scalar.activation(
                                        st["sig"][:], st["tmp"][:],
                                        mybir.ActivationFunctionType.Copy,
                                        bias=0.5, scale=0.5)
                                nc.scalar.activation(
                                    st["tg"][:], ps[3][:],
                                    mybir.ActivationFunctionType.Tanh)
                            # cell update
                            sig = st["sig"]
                            nc.vector.tensor_tensor(out=st["pr"][:],
                                                    in0=sig[:, :2 * B],
                                                    in1=st["tg"][:],
                                                    op=mybir.AluOpType.mult)
                            nc.vector.tensor_tensor(out=st["c"][:],
                                                    in0=sig[:, 2 * B:4 * B],
                                                    in1=st["c"][:],
                                                    op=mybir.AluOpType.mult)
                            nc.vector.tensor_tensor(out=st["c"][:],
                                                    in0=st["c"][:],
                                                    in1=st["pr"][:],
                                                    op=mybir.AluOpType.add)
                            nc.scalar.activation(st["tc"][:], st["c"][:],
                                                 mybir.ActivationFunctionType.Tanh)
                            hdst = st["hbf"] if t + 1 < WBF else st["h"]
                            nc.vector.tensor_tensor(out=hdst[:],
                                                    in0=sig[:, 4 * B:6 * B],
                                                    in1=st["tc"][:],
                                                    op=mybir.AluOpType.mult)
                            if d == 0:
                                if t >= W:
                                    hoff = t - (W - WV)
                                    for k in range(2):
                                        nc.vector.tensor_copy(
                                            hfull[0, k][:, hoff::L][:, :B],
                                            st["h"][:, k * B:(k + 1) * B])
                                elif t >= W - WV:
                                    col = t - (W - WV)
                                    hsrc = st["hbf"] if t + 1 < WBF else st["h"]
                                    for k in range(2):
                                        nc.vector.tensor_copy(
                                            hfull[0, k][:, col:col + 1],
                                            hsrc[:, k * B:k * B + 1])
                            else:
                                if t >= W:
                                    hoff = (L - 1) - (t - W)
                                    for k in range(2):
                                        nc.vector.tensor_copy(
                                            hfull[1, k][:, hoff::L][:, :B],
                                            st["h"][:, k * B:(k + 1) * B])
                        # overlap fp32 pre with the bf16 head
                        if t < WBF and fp32_pre_jobs:
                            for _ in range(2):
                                if fp32_pre_jobs:
                                    d_, m_, ncol_, c0_ = fp32_pre_jobs.pop(0)
                                    pre_mm(d_, m_, ncol_, c0_, False)
                    assert not fp32_pre_jobs

            # ---- viterbi constants + pools (after LSTM sbuf freed) ----
            with tc.tile_pool(name="vit", bufs=1) as vpool:
                wtag_sb = []
                for k in range(4):
                    t_ = vpool.tile([P, T], f32, tag=f"wtag{k}", name=f"wtag{k}")
                    nc.sync.dma_start(out=t_[:], in_=wtagT[k])
                    wtag_sb.append(t_)
                btag_sb = vpool.tile([P, VSTEP * T], f32)
                nc.sync.dma_start(out=btag_sb[:], in_=btag_rep[:])
                trR_sb = vpool.tile([P, T * T], f32)
                nc.sync.dma_start(out=trR_sb[:], in_=transR[:])
                vmask_sb = vpool.tile([P, WV], i32)
                nc.sync.dma_start(out=vmask_sb[:], in_=vmask[:])
                iot_sb = vpool.tile([P, L * T * T], f32)
                nc.sync.dma_start(out=iot_sb[:], in_=iot[:])
                vinit_sb = vpool.tile([P, T], f32)
                nc.sync.dma_start(out=vinit_sb[:], in_=vinit_rep[:])

                # ---- phase D+E: feats (PE) pipelined with viterbi (DVE) ----
                featsv = vpool.tile([P, VSTEP * T], f32)
                vhist = vpool.tile([P, VSTEP * T], f32)
                vvhist = vpool.tile([P, L * T], f32)
                schist = vpool.tile([P, L * T * T], f32)
                scscr = vpool.tile([P, T * T], f32)
                vmscr = vpool.tile([P, T], f32)
                trR3 = trR_sb[:].rearrange("p (a b) -> p a b", b=T)
                vprev = vinit_sb[:]
                sc_e = nc.enter_named_scope("viterbi", False)
                with tc.tile_pool(name="psD", bufs=1, space="PSUM") as psD:
                    for t in range(VSTEP):
                        fv = psD.tile([P, T], f32, space="PSUM", tag="fvt",
                                      bufs=4, name="fvt")
                        for k4 in range(4):
                            d, k = divmod(k4, 2)
                            lhs = hfull[d, k][:, t::L][:, :P]
                            nc.tensor.matmul(fv[:], lhs, wtag_sb[k4][:],
                                             start=(k4 == 0), stop=(k4 == 3))
                        nc.vector.tensor_tensor(
                            out=featsv[:, t * T:(t + 1) * T], in0=fv[:],
                            in1=btag_sb[:, t * T:(t + 1) * T],
                            op=mybir.AluOpType.add)
                        real = t >= WV
                        sc_ap = (schist[:, (t - WV) * T * T:(t - WV + 1) * T * T]
                                 if real else scscr[:])
                        sc3 = sc_ap.rearrange("p (a b) -> p a b", b=T)
                        nc.vector.tensor_tensor(out=sc3, in0=bc_mid(vprev, T),
                                                in1=trR3, op=mybir.AluOpType.add)
                        vv_ap = (vvhist[:, (t - WV) * T:(t - WV + 1) * T]
                                 if real else vmscr[:])
                        nc.vector.tensor_reduce(out=vv_ap, in_=sc3,
                                                axis=mybir.AxisListType.X,
                                                op=mybir.AluOpType.max)
                        vdst = vhist[:, t * T:(t + 1) * T]
                        nc.vector.tensor_tensor(out=vdst, in0=vv_ap,
                                                in1=featsv[:, t * T:(t + 1) * T],
                                                op=mybir.AluOpType.add)
                        if t < WV:
                            mb = vmask_sb[:, t:t + 1]
                            mask_bc = bass.AP(mb.tensor, mb.offset,
                                              [mb.ap[0], [0, T]])
                            nc.vector.copy_predicated(vdst, mask_bc, vprev)
                        vprev = vdst
                nc.sync.dma_start(out=o_feats[:], in_=featsv[:])
                # ---- phase F: backpointers ----
                mask = vpool.tile([P, L * T * T], f32)
                sch3 = schist[:].rearrange("p (a b) -> p a b", b=T)
                vvb = bc_last(vvhist[:], T)
                nc.vector.tensor_tensor(out=mask[:].rearrange("p (a b) -> p a b", b=T),
                                        in0=sch3, in1=vvb,
                                        op=mybir.AluOpType.is_equal)
                nc.vector.tensor_tensor(out=mask[:], in0=mask[:], in1=iot_sb[:],
                                        op=mybir.AluOpType.mult)
                r96 = vpool.tile([P, L * T], f32)
                nc.vector.tensor_reduce(out=r96[:],
                                        in_=mask[:].rearrange("p (a b) -> p a b", b=T),
                                        axis=mybir.AxisListType.X,
                                        op=mybir.AluOpType.max)
                bp_sb = vpool.tile([P, L * T], f32)
                nc.scalar.activation(bp_sb[:], r96[:],
                                     mybir.ActivationFunctionType.Copy,
                                     bias=float(T), scale=-1.0)
                nc.sync.dma_start(out=o_bptr[:], in_=bp_sb[:])
                nc.sync.dma_start(out=o_vhist[:], in_=vhist[:])
                nc.leave_named_scope("viterbi", sc_e[0], False)
    nc.compile()
    return nc


def _prep_static(emb_table, w_ih_f, w_hh_f, b_f, w_ih_b, w_hh_b, b_b,
                 w_tag, b_tag, transitions):
    """Host-side weight reordering/padding shared by all cores."""
    P = 128
    perm = np.r_[0:256, 256:512, 768:1024, 512:768]  # [i, f, o, g]
    out = {}
    out["emb"] = np.ascontiguousarray(emb_table.astype(np.float32))

    wihT = np.zeros((2, 3, P, G4), np.float32)
    whhT = np.zeros((2, 2, P, G4), np.float32)
    biasr = np.zeros((P, 16), np.float32)
    for d, (wi, wh, bb_) in enumerate(((w_ih_f, w_hh_f, b_f),
                                       (w_ih_b, w_hh_b, b_b))):
        wiT = wi[perm].T.astype(np.float32)          # [300, 1024]
        wiTp = np.zeros((384, G4), np.float32)
        wiTp[:E] = wiT
        for k in range(3):
            wihT[d, k] = wiTp[128 * k:128 * (k + 1)]
        whT = wh[perm].T.astype(np.float32)          # [256, 1024]
        for k in range(2):
            whhT[d, k] = whT[128 * k:128 * (k + 1)]
        biasr[:, d * 8:(d + 1) * 8] = bb_[perm].astype(np.float32).reshape(8, P).T
    out["wihT"], out["whhT"], out["biasr"] = wihT, whhT, biasr

    wtT = w_tag.T.astype(np.float32)                 # [512, 24]
    out["wtagT"] = wtT.reshape(4, P, T).copy()

    out["btag_rep"] = np.tile(b_tag.astype(np.float32)[None, :],
                              (P, VSTEP)).reshape(P, VSTEP * T).copy()
    out["transR"] = np.tile(transitions.astype(np.float32).reshape(1, T * T),
                            (P, 1)).copy()
    iot = np.tile((float(T) - np.arange(T, dtype=np.float32))[None, None, :],
                  (P, L * T, 1)).reshape(P, L * T * T)
    out["iot"] = np.ascontiguousarray(iot)
    vinit = np.full(T, NEG, np.float32)
    vinit[START] = 0.0
    out["vinit_rep"] = np.tile(vinit[None, :], (P, 1)).copy()
    return out


def kernel(sentence, emb_table, w_ih_f, w_hh_f, b_f, w_ih_b, w_hh_b, b_b,
           w_tag, b_tag, transitions):
    sentence = np.asarray(sentence)
    sent = sentence.astype(np.int64)
    trans = np.asarray(transitions, np.float32)

    if "nc" not in _PROG_CACHE:
        _PROG_CACHE["nc"] = _build_program()
    nc = _PROG_CACHE["nc"]

    static = _prep_static(np.asarray(emb_table), np.asarray(w_ih_f),
                          np.asarray(w_hh_f), np.asarray(b_f),
                          np.asarray(w_ih_b), np.asarray(w_hh_b),
                          np.asarray(b_b), np.asarray(w_tag),
                          np.asarray(b_tag), trans)

    in_maps = []
    for c in range(NCORES):
        m = dict(static)
        pos = np.arange(NPOS, dtype=np.int64) + (SPAN * c - XOFF)
        tokc = np.where((pos >= 0) & (pos < S), sent[np.clip(pos, 0, S - 1)],
                        OOB).astype(np.int32)
        m["tok"] = tokc
        vm = np.zeros((128, WV), np.int32)
        if c == 0:
            for b in range(WV // L):
                vm[b, :WV - L * b] = 1
        m["vmask"] = vm
        in_maps.append(m)

    trace = bool(os.environ.get("BASS_TRACE_KERNEL"))
    if trace:
        import ntff_shim  # noqa: F401
    res = run_bass_kernel_spmd(nc, in_maps, list(range(NCORES)), trace=trace)
    _PROG_CACHE["last_res"] = res

    # host postprocessing: backtrace + path score
    bp = np.zeros((S, T), np.int32)
    feats = np.zeros((S, T), np.float32)
    for c in range(NCORES):
        r = res.results[c]
        bpc = r["bptrs"].reshape(128, L, T)      # [chunk, t, to]
        fvc = r["featsv"].reshape(128, VSTEP, T)[:, WV:, :]
        bp[SPAN * c:SPAN * (c + 1)] = bpc.reshape(SPAN, T)
        feats[SPAN * c:SPAN * (c + 1)] = fvc.reshape(SPAN, T)

    v_end = res.results[NCORES - 1]["vhist"].reshape(128, VSTEP, T)[-1, -1]
    term = (v_end + trans[STOP]).astype(np.float32)
    best = int(np.argmax(term))
    path = np.zeros(S, np.int32)
    tag = best
    for t in range(S - 1, -1, -1):
        path[t] = tag
        tag = bp[t, tag]

    sc = np.float32(0.0)
    prev = START
    for t in range(S):
        sc = np.float32(np.float32(sc + trans[path[t], prev]) + feats[t, path[t]])
        prev = path[t]
    sc = np.float32(sc + trans[STOP, path[-1]])
    return np.float32(sc), path.astype(np.int32)


# revision 19
# speedup vs baseline: 1.1776x; 1.0650x over previous
"""BiLSTM-CRF Viterbi decode on 8 Trainium2 NeuronCores (Bass/Tile).

Strategy (self-contained, shapes hardcoded):
  - 8 cores, core c owns sentence positions [512c, 512(c+1)).
  - Embedding table replicated; each core indirect-DMA-gathers the 640
    token rows covering [512c-64, 512c+576) (out-of-range -> zero rows).
  - pre = x @ w_ih.T + b precomputed on PE for both directions.
  - LSTM recurrence: chunked with zero-init warmup. Per core two
    interleaved streams: forward (128 chunks x 4 tokens, warmup 48) and
    backward (136 chunks covering [512c-32, 512(c+1)), warmup 48).
    Gates are computed on PE (fp32), sigmoid via 0.5*tanh(x/2)+0.5 (the
    Tanh LUT is ~1 ulp), cell update on DVE. Warmup makes each chunk's
    state agree with the sequential scan to ~1 ulp (contraction of the
    LSTM map); boundary cores are exact via zero-x padding.
  - feats = h @ w_tag.T + b_tag computed directly in [chunk, (t, tag)]
    layout (144 small matmuls with column-strided stationary operands).
  - Viterbi: 128 chunks x 4 positions per core, warmup 32 from vinit
    (max-plus coalescence); core-0 short chunks are made exact with
    per-step copy_predicated restores. Backpointers recovered in one
    batched pass (is_equal + reversed-iota + max-reduce).
  - Host: backtrace over device backpointers, score re-accumulated in
    fp32 along the path (bitwise-faithful to the reference recursion).
"""
import os
import sys

for _p in ("/opt/trn_rl_repo", "/root/.axon_site/_ro/trn_rl_repo"):
    if os.path.isdir(_p) and _p not in sys.path:
        sys.path.append(_p)

import numpy as np

import concourse.bass as bass
import concourse.bacc as bacc
import concourse.mybir as mybir
import concourse.tile as tile
from concourse.bass_utils import run_bass_kernel_spmd
from concourse.masks import make_identity

f32 = mybir.dt.float32
i32 = mybir.dt.int32

# problem constants
S = 4096
V = 100000
E = 300
HH = 256
G4 = 1024
T = 24
START, STOP = 22, 23
NEG = -10000.0

NCORES = 8
SPAN = S // NCORES          # 512 positions per core
L = 4                       # tokens per LSTM/viterbi chunk
W = 32                      # LSTM warmup steps
WBF = 20                    # of which: bf16 hh-matmul head
WV = 16                     # viterbi warmup steps
BF = SPAN // L              # 128 forward chunks
BB = (SPAN + WV) // L       # 136 backward chunks (cover 32-pos left spill)
NSTEP = L + W               # 52 LSTM steps per stream
VSTEP = L + WV              # 36 viterbi steps
NPOS = 640                  # gathered x / pre positions: [512c-64, 512c+576)
XOFF = 64                   # position p -> pre column p - 512c + XOFF
HFULL = SPAN + WV           # 544 h columns: [512c-32, 512(c+1))
OOB = 1 << 20

_PROG_CACHE = {}


def _build_program():
    nc = bacc.Bacc("TRN2", target_bir_lowering=False, debug=False,
                   num_devices=NCORES)
    P = 128
    bf16 = mybir.dt.bfloat16

    def din(name, shape, dt=f32):
        return nc.dram_tensor(name, shape, dt, kind="ExternalInput").ap()

    def dout(name, shape, dt=f32):
        return nc.dram_tensor(name, shape, dt, kind="ExternalOutput").ap()

    emb = din("emb", [V, E])
    tok = din("tok", [NPOS], i32)
    wihT = din("wihT", [2, 3, P, G4])
    whhT = din("whhT", [2, 2, P, G4])
    biasr = din("biasr", [P, 16])
    wtagT = din("wtagT", [4, P, T])
    btag_rep = din("btag_rep", [P, VSTEP * T])
    transR = din("transR", [P, T * T])
    vmask = din("vmask", [P, WV], i32)
    iot = din("iot", [P, L * T * T])
    vinit_rep = din("vinit_rep", [P, T])

    o_bptr = dout("bptrs", [P, L * T])
    o_vhist = dout("vhist", [P, VSTEP * T])
    o_feats = dout("featsv", [P, VSTEP * T])

    def bc_mid(ap2, n):
        return bass.AP(ap2.tensor, ap2.offset, [ap2.ap[0], [0, n], ap2.ap[1]])

    def bc_last(ap2, n):
        return bass.AP(ap2.tensor, ap2.offset, [ap2.ap[0], ap2.ap[1], [0, n]])

    with tile.TileContext(nc) as tc:
        with (
            tc.tile_pool(name="const", bufs=1) as cpool,
            tc.tile_pool(name="state", bufs=1) as spool,
        ):
            ident = cpool.tile([P, P], f32)
            make_identity(nc, ident[:])
            ident_bf = cpool.tile([P, P], bf16, name="identbf")
            nc.vector.tensor_copy(ident_bf[:], ident[:])

            hfull = {}
            for d in range(2):
                for k in range(2):
                    t_ = spool.tile([P, HFULL], f32, tag=f"hfull{d}{k}", name=f"hfull{d}{k}")
                    hfull[d, k] = t_
            pre_f = spool.tile([P, 8 * NPOS], f32, tag="pref", name="pref")
            pre_pair = {}
            for j in range(4):
                t_ = spool.tile([P, 2 * NPOS], f32, tag=f"pre1{j}", name=f"pre1{j}")
                pre_pair[1, j] = t_
            pre_f_bf = spool.tile([P, 8 * NPOS], bf16, tag="prefbf", name="prefbf")
            pre_bf1 = {}
            for j in range(4):
                t_ = spool.tile([P, 2 * NPOS], bf16, tag=f"preb1{j}", name=f"preb1{j}")
                pre_bf1[j] = t_

            with tc.tile_pool(name="wih", bufs=1) as wpool, \
                    tc.tile_pool(name="psC", bufs=1, space="PSUM") as psC:
                # ---- phase A: gather x, transpose, casts ----
                with tc.tile_pool(name="phA", bufs=1) as apool:
                    tok_sb = apool.tile([P, 5], i32)
                    nc.sync.dma_start(out=tok_sb[:],
                                      in_=tok.rearrange("(j p) -> p j", p=P))
                    xg = []
                    for j in range(5):
                        xt = apool.tile([P, E], f32, tag=f"x{j}", name=f"x{j}")
                        nc.vector.memset(xt[:], 0.0)
                        nc.gpsimd.indirect_dma_start(
                            out=xt[:], out_offset=None, in_=emb[:, :],
                            in_offset=bass.IndirectOffsetOnAxis(
                                ap=tok_sb[:, j:j + 1], axis=0),
                            bounds_check=V - 1, oob_is_err=False)
                        xg.append(xt)
                    xT = []
                    xT_bf = []
                    for k in range(3):
                        t_ = wpool.tile([P, NPOS], f32, tag=f"xT{k}", name=f"xT{k}")
                        if k == 2:
                            nc.vector.memset(t_[:], 0.0)
                        xT.append(t_)
                        tb_ = wpool.tile([P, NPOS], bf16, tag=f"xTb{k}", name=f"xTb{k}")
                        if k == 2:
                            nc.vector.memset(tb_[:], 0)
                        xT_bf.append(tb_)
                    for j in range(5):
                        for k in range(3):
                            ecols = min(128, E - 128 * k)
                            tp = psC.tile([P, P], f32, space="PSUM", tag="pm",
                                          bufs=2, name="tp")
                            nc.tensor.transpose(tp[:ecols, :],
                                                xg[j][:, 128 * k:128 * k + ecols],
                                                ident[:])
                            nc.vector.tensor_copy(
                                xT[k][:ecols, j * P:(j + 1) * P], tp[:ecols, :])
                            nc.vector.tensor_copy(
                                xT_bf[k][:ecols, j * P:(j + 1) * P], tp[:ecols, :])
                    bias_sb = cpool.tile([P, 16], f32)
                    nc.sync.dma_start(out=bias_sb[:], in_=biasr[:])
                    wih_sb = {}
                    wih_bf = {}
                    for d in range(2):
                        for k in range(3):
                            t_ = wpool.tile([P, G4], f32, tag=f"wih{d}{k}", name=f"wih{d}{k}")
                            nc.sync.dma_start(out=t_[:], in_=wihT[d, k])
                            wih_sb[d, k] = t_
                            tb_ = wpool.tile([P, G4], bf16, tag=f"wihb{d}{k}", name=f"wihb{d}{k}")
                            nc.vector.tensor_copy(tb_[:], t_[:])
                            wih_bf[d, k] = tb_

                # ---- phase B-bf16: bf16 pre for the warmup head ----
                def pre_mm(d, m, ncol, c0, use_bf):
                    pm = psC.tile([P, 512], f32, space="PSUM",
                                  tag="pm", bufs=2, name="pm")
                    wsrc = wih_bf if use_bf else wih_sb
                    xsrc = xT_bf if use_bf else xT
                    for k in range(3):
                        nc.tensor.matmul(
                            pm[:, :ncol],
                            wsrc[d, k][:, 128 * m:128 * (m + 1)],
                            xsrc[k][:, c0:c0 + ncol],
                            start=(k == 0), stop=(k == 2))
                    if use_bf:
                        if d == 0:
                            dst = pre_f_bf[:, m * NPOS + c0:m * NPOS + c0 + ncol]
                        else:
                            j, half = divmod(m, 2)
                            dst = pre_bf1[j][:, half * NPOS + c0:
                                             half * NPOS + c0 + ncol]
                    else:
                        if d == 0:
                            dst = pre_f[:, m * NPOS + c0:m * NPOS + c0 + ncol]
                        else:
                            j, half = divmod(m, 2)
                            dst = pre_pair[1, j][:, half * NPOS + c0:
                                                 half * NPOS + c0 + ncol]
                    nc.vector.tensor_scalar_add(
                        dst, pm[:, :ncol], bias_sb[:, d * 8 + m:d * 8 + m + 1])

                for d in range(2):
                    for m in range(8):
                        for ncol, c0 in ((512, 0), (128, 512)):
                            pre_mm(d, m, ncol, c0, True)
                fp32_pre_jobs = [(d, m, ncol, c0)
                                 for d in range(2) for m in range(8)
                                 for ncol, c0 in ((512, 0), (128, 512))]

                whh_sb = {}
                whh_bf = {}
                for d in range(2):
                    for k in range(2):
                        t_ = cpool.tile([P, G4], f32, tag=f"whh{d}{k}", name=f"whh{d}{k}")
                        nc.sync.dma_start(out=t_[:], in_=whhT[d, k])
                        whh_sb[d, k] = t_
                        tb_ = cpool.tile([P, G4], bf16, tag=f"whb{d}{k}", name=f"whb{d}{k}")
                        nc.vector.tensor_copy(tb_[:], t_[:])
                        whh_bf[d, k] = tb_

                # ---- phase C: LSTM streams ----
                streams = [dict(d=0, B=BF), dict(d=1, B=BB)]
                with nc.named_scope("lstm"):
                    for st in streams:
                        B = st["B"]
                        d = st["d"]
                        st["h"] = spool.tile([P, 2 * B], f32, tag=f"h{d}", name=f"hcur{d}")
                        st["c"] = spool.tile([P, 2 * B], f32, tag=f"c{d}", name=f"ccur{d}")
                        st["tmp"] = spool.tile([P, 6 * B], f32, tag=f"tm{d}", name=f"tmp{d}")
                        st["sig"] = spool.tile([P, 6 * B], f32, tag=f"sg{d}", name=f"sig{d}")
                        st["tg"] = spool.tile([P, 2 * B], f32, tag=f"tg{d}", name=f"tgg{d}")
                        st["tc"] = spool.tile([P, 2 * B], f32, tag=f"tc{d}", name=f"tcc{d}")
                        st["pr"] = spool.tile([P, 2 * B], f32, tag=f"pr{d}", name=f"prd{d}")
                        st["hbf"] = spool.tile([P, 2 * B], bf16, tag=f"hb{d}", name=f"hbf{d}")
                        nc.vector.memset(st["h"][:], 0.0)
                        nc.vector.memset(st["c"][:], 0.0)
                        nc.vector.memset(st["hbf"][:], 0)

                    for t in range(NSTEP):
                        for st in streams:
                            d, B = st["d"], st["B"]
                            bf = t < WBF
                            wsel = whh_bf if bf else whh_sb
                            hsel = st["hbf"] if bf else st["h"]
                            if d == 0:
                                poff = (XOFF - W) + t
                                ps = [psC.tile([P, 4 * B], f32, space="PSUM",
                                               tag=f"psf{jj}", name=f"psf{jj}")
                                      for jj in range(2)]
                                for m in range(8):
                                    half, mm = divmod(m, 4)
                                    dstp = ps[half][:, mm * B:(mm + 1) * B]
                                    if bf:
                                        pslice = pre_f_bf[:, m * NPOS + poff::L][:, :B]
                                        nc.tensor.matmul(dstp, ident_bf[:], pslice,
                                                         start=True, stop=False)
                                    for k in range(2):
                                        nc.tensor.matmul(
                                            dstp,
                                            wsel[0, k][:, 128 * m:128 * (m + 1)],
                                            hsel[:, k * B:(k + 1) * B],
                                            start=(k == 0 and not bf),
                                            stop=(k == 1))
                                if not bf:
                                    for half in range(2):
                                        pslice = pre_f[:].rearrange(
                                            "p (m n) -> p m n", m=8)[:, 4 * half:4 * half + 4,
                                                                     poff::L][:, :, :B]
                                        nc.vector.tensor_tensor(
                                            out=ps[half][:].rearrange("p (m b) -> p m b", m=4),
                                            in0=ps[half][:].rearrange("p (m b) -> p m b", m=4),
                                            in1=pslice, op=mybir.AluOpType.add)
                                if bf:
                                    nc.scalar.activation(
                                        st["sig"][:, :4 * B], ps[0][:],
                                        mybir.ActivationFunctionType.Sigmoid)
                                    nc.scalar.activation(
                                        st["sig"][:, 4 * B:6 * B], ps[1][:, :2 * B],
                                        mybir.ActivationFunctionType.Sigmoid)
                                else:
                                    nc.scalar.activation(
                                        st["tmp"][:, :4 * B], ps[0][:],
                                        mybir.ActivationFunctionType.Tanh,
                                        bias=0.0, scale=0.5)
                                    nc.scalar.activation(
                                        st["tmp"][:, 4 * B:6 * B], ps[1][:, :2 * B],
                                        mybir.ActivationFunctionType.Tanh,
                                        bias=0.0, scale=0.5)
                                    nc.scalar.activation(
                                        st["sig"][:], st["tmp"][:],
                                        mybir.ActivationFunctionType.Copy,
                                        bias=0.5, scale=0.5)
                                nc.scalar.activation(
                                    st["tg"][:], ps[1][:, 2 * B:4 * B],
                                    mybir.ActivationFunctionType.Tanh)
                            else:
                                poff = (XOFF - WV) + (L - 1) + W - t
                                ps = [psC.tile([P, 2 * B], f32, space="PSUM",
                                               tag=f"psb{jj}", name=f"psb{jj}")
                                      for jj in range(4)]
                                for j in range(4):
                                    for half in range(2):
                                        m = 2 * j + half
                                        dstp = ps[j][:, half * B:(half + 1) * B]
                                        if bf:
                                            pslice = pre_bf1[j][:, half * NPOS + poff::L][:, :B]
                                            nc.tensor.matmul(dstp, ident_bf[:], pslice,
                                                             start=True, stop=False)
                                        for k in range(2):
                                            nc.tensor.matmul(
                                                dstp,
                                                wsel[1, k][:, 128 * m:128 * (m + 1)],
                                                hsel[:, k * B:(k + 1) * B],
                                                start=(k == 0 and not bf),
                                                stop=(k == 1))
                                if not bf:
                                    for j in range(4):
                                        pslice = pre_pair[1, j][:].rearrange(
                                            "p (m n) -> p m n", m=2)[:, :, poff::L][:, :, :B]
                                        nc.vector.tensor_tensor(
                                            out=ps[j][:].rearrange("p (m b) -> p m b", m=2),
                                            in0=ps[j][:].rearrange("p (m b) -> p m b", m=2),
                                            in1=pslice, op=mybir.AluOpType.add)
                                if bf:
                                    for j in range(3):
                                        nc.scalar.activation(
                                            st["sig"][:, j * 2 * B:(j + 1) * 2 * B],
                                            ps[j][:],
                                            mybir.ActivationFunctionType.Sigmoid)
                                else:
                                    for j in range(3):
                                        nc.scalar.activation(
                                            st["tmp"][:, j * 2 * B:(j + 1) * 2 * B],
                                            ps[j][:],
                                            mybir.ActivationFunctionType.Tanh,
                                            bias=0.0, scale=0.5)
                                    nc.scalar.activation(
                                        st["sig"][:], st["tmp"][:],
                                        mybir.ActivationFunctionType.Copy,
                                        bias=0.5, scale=0.5)
                                nc.scalar.activation(
                                    st["tg"][:], ps[3][:],
                                    mybir.ActivationFunctionType.Tanh)
                            # cell update
                            sig = st["sig"]
                            nc.vector.tensor_tensor(out=st["pr"][:],
                                                    in0=sig[:, :2 * B],
                                                    in1=st["tg"][:],
                                                    op=mybir.AluOpType.mult)
                            nc.vector.tensor_tensor(out=st["c"][:],
                                                    in0=sig[:, 2 * B:4 * B],
                                                    in1=st["c"][:],
                                                    op=mybir.AluOpType.mult)
                            nc.vector.tensor_tensor(out=st["c"][:],
                                                    in0=st["c"][:],
                                                    in1=st["pr"][:],
                                                    op=mybir.AluOpType.add)
                            nc.scalar.activation(st["tc"][:], st["c"][:],
                                                 mybir.ActivationFunctionType.Tanh)
                            hdst = st["hbf"] if t + 1 < WBF else st["h"]
                            nc.vector.tensor_tensor(out=hdst[:],
                                                    in0=sig[:, 4 * B:6 * B],
                                                    in1=st["tc"][:],
                                                    op=mybir.AluOpType.mult)
                            if d == 0:
                                if t >= W:
                                    hoff = t - (W - WV)
                                    for k in range(2):
                                        nc.vector.tensor_copy(
                                            hfull[0, k][:, hoff::L][:, :B],
                                            st["h"][:, k * B:(k + 1) * B])
                                elif t >= W - WV:
                                    col = t - (W - WV)
                                    hsrc = st["hbf"] if t + 1 < WBF else st["h"]
                                    for k in range(2):
                                        nc.vector.tensor_copy(
                                            hfull[0, k][:, col:col + 1],
                                            hsrc[:, k * B:k * B + 1])
                            else:
                                if t >= W:
                                    hoff = (L - 1) - (t - W)
                                    for k in range(2):
                                        nc.vector.tensor_copy(
                                            hfull[1, k][:, hoff::L][:, :B],
                                            st["h"][:, k * B:(k + 1) * B])
                        # overlap fp32 pre with the bf16 head
                        if t < WBF and fp32_pre_jobs:
                            for _ in range(2):
                                if fp32_pre_jobs:
                                    d_, m_, ncol_, c0_ = fp32_pre_jobs.pop(0)
                                    pre_mm(d_, m_, ncol_, c0_, False)
                    assert not fp32_pre_jobs

            # ---- viterbi constants + pools (after LSTM sbuf freed) ----
            with tc.tile_pool(name="vit", bufs=1) as vpool:
                wtag_sb = []
                for k in range(4):
                    t_ = vpool.tile([P, T], f32, tag=f"wtag{k}", name=f"wtag{k}")
                    nc.sync.dma_start(out=t_[:], in_=wtagT[k])
                    wtag_sb.append(t_)
                btag_sb = vpool.tile([P, VSTEP * T], f32)
                nc.sync.dma_start(out=btag_sb[:], in_=btag_rep[:])
                trR_sb = vpool.tile([P, T * T], f32)
                nc.sync.dma_start(out=trR_sb[:], in_=transR[:])
                vmask_sb = vpool.tile([P, WV], i32)
                nc.sync.dma_start(out=vmask_sb[:], in_=vmask[:])
                iot_sb = vpool.tile([P, L * T * T], f32)
                nc.sync.dma_start(out=iot_sb[:], in_=iot[:])
                vinit_sb = vpool.tile([P, T], f32)
                nc.sync.dma_start(out=vinit_sb[:], in_=vinit_rep[:])

                # ---- phase D+E: feats (PE) pipelined with viterbi (DVE) ----
                featsv = vpool.tile([P, VSTEP * T], f32)
                vhist = vpool.tile([P, VSTEP * T], f32)
                vvhist = vpool.tile([P, L * T], f32)
                schist = vpool.tile([P, L * T * T], f32)
                scscr = vpool.tile([P, T * T], f32)
                vmscr = vpool.tile([P, T], f32)
                trR3 = trR_sb[:].rearrange("p (a b) -> p a b", b=T)
                vprev = vinit_sb[:]
                sc_e = nc.enter_named_scope("viterbi", False)
                with tc.tile_pool(name="psD", bufs=1, space="PSUM") as psD:
                    for t in range(VSTEP):
                        fv = psD.tile([P, T], f32, space="PSUM", tag="fvt",
                                      bufs=4, name="fvt")
                        for k4 in range(4):
                            d, k = divmod(k4, 2)
                            lhs = hfull[d, k][:, t::L][:, :P]
                            nc.tensor.matmul(fv[:], lhs, wtag_sb[k4][:],
                                             start=(k4 == 0), stop=(k4 == 3))
                        nc.vector.tensor_tensor(
                            out=featsv[:, t * T:(t + 1) * T], in0=fv[:],
                            in1=btag_sb[:, t * T:(t + 1) * T],
                            op=mybir.AluOpType.add)
                        real = t >= WV
                        sc_ap = (schist[:, (t - WV) * T * T:(t - WV + 1) * T * T]
                                 if real else scscr[:])
                        sc3 = sc_ap.rearrange("p (a b) -> p a b", b=T)
                        nc.vector.tensor_tensor(out=sc3, in0=bc_mid(vprev, T),
                                                in1=trR3, op=mybir.AluOpType.add)
                        vv_ap = (vvhist[:, (t - WV) * T:(t - WV + 1) * T]
                                 if real else vmscr[:])
                        nc.vector.tensor_reduce(out=vv_ap, in_=sc3,
                                                axis=mybir.AxisListType.X,
                                                op=mybir.AluOpType.max)
                        vdst = vhist[:, t * T:(t + 1) * T]
                        nc.vector.tensor_tensor(out=vdst, in0=vv_ap,
                                                in1=featsv[:, t * T:(t + 1) * T],
                                                op=mybir.AluOpType.add)
                        if t < WV:
                            mb = vmask_sb[:, t:t + 1]
                            mask_bc = bass.AP(mb.tensor, mb.offset,
                                              [mb.ap[0], [0, T]])
                            nc.vector.copy_predicated(vdst, mask_bc, vprev)
                        vprev = vdst
                nc.sync.dma_start(out=o_feats[:], in_=featsv[:])
                # ---- phase F: backpointers ----
                mask = vpool.tile([P, L * T * T], f32)
                sch3 = schist[:].rearrange("p (a b) -> p a b", b=T)
                vvb = bc_last(vvhist[:], T)
                nc.vector.tensor_tensor(out=mask[:].rearrange("p (a b) -> p a b", b=T),
                                        in0=sch3, in1=vvb,
                                        op=mybir.AluOpType.is_equal)
                nc.vector.tensor_tensor(out=mask[:], in0=mask[:], in1=iot_sb[:],
                                        op=mybir.AluOpType.mult)
                r96 = vpool.tile([P, L * T], f32)
                nc.vector.tensor_reduce(out=r96[:],
                                        in_=mask[:].rearrange("p (a b) -> p a b", b=T),
                                        axis=mybir.AxisListType.X,
                                        op=mybir.AluOpType.max)
                bp_sb = vpool.tile([P, L * T], f32)
                nc.scalar.activation(bp_sb[:], r96[:],
                                     mybir.ActivationFunctionType.Copy,
                                     bias=float(T), scale=-1.0)
                nc.sync.dma_start(out=o_bptr[:], in_=bp_sb[:])
                nc.sync.dma_start(out=o_vhist[:], in_=vhist[:])
                nc.leave_named_scope("viterbi", sc_e[0], False)
    nc.compile()
    return nc


def _prep_static(emb_table, w_ih_f, w_hh_f, b_f, w_ih_b, w_hh_b, b_b,
                 w_tag, b_tag, transitions):
    """Host-side weight reordering/padding shared by all cores."""
    P = 128
    perm = np.r_[0:256, 256:512, 768:1024, 512:768]  # [i, f, o, g]
    out = {}
    out["emb"] = np.ascontiguousarray(emb_table.astype(np.float32))

    wihT = np.zeros((2, 3, P, G4), np.float32)
    whhT = np.zeros((2, 2, P, G4), np.float32)
    biasr = np.zeros((P, 16), np.float32)
    for d, (wi, wh, bb_) in enumerate(((w_ih_f, w_hh_f, b_f),
                                       (w_ih_b, w_hh_b, b_b))):
        wiT = wi[perm].T.astype(np.float32)          # [300, 1024]
        wiTp = np.zeros((384, G4), np.float32)
        wiTp[:E] = wiT
        for k in range(3):
            wihT[d, k] = wiTp[128 * k:128 * (k + 1)]
        whT = wh[perm].T.astype(np.float32)          # [256, 1024]
        for k in range(2):
            whhT[d, k] = whT[128 * k:128 * (k + 1)]
        biasr[:, d * 8:(d + 1) * 8] = bb_[perm].astype(np.float32).reshape(8, P).T
    out["wihT"], out["whhT"], out["biasr"] = wihT, whhT, biasr

    wtT = w_tag.T.astype(np.float32)                 # [512, 24]
    out["wtagT"] = wtT.reshape(4, P, T).copy()

    out["btag_rep"] = np.tile(b_tag.astype(np.float32)[None, :],
                              (P, VSTEP)).reshape(P, VSTEP * T).copy()
    out["transR"] = np.tile(transitions.astype(np.float32).reshape(1, T * T),
                            (P, 1)).copy()
    iot = np.tile((float(T) - np.arange(T, dtype=np.float32))[None, None, :],
                  (P, L * T, 1)).reshape(P, L * T * T)
    out["iot"] = np.ascontiguousarray(iot)
    vinit = np.full(T, NEG, np.float32)
    vinit[START] = 0.0
    out["vinit_rep"] = np.tile(vinit[None, :], (P, 1)).copy()
    return out


def kernel(sentence, emb_table, w_ih_f, w_hh_f, b_f, w_ih_b, w_hh_b, b_b,
           w_tag, b_tag, transitions):
    sentence = np.asarray(sentence)
    sent = sentence.astype(np.int64)
    trans = np.asarray(transitions, np.float32)

    if "nc" not in _PROG_CACHE:
        _PROG_CACHE["nc"] = _build_program()
    nc = _PROG_CACHE["nc"]

    static = _prep_static(np.asarray(emb_table), np.asarray(w_ih_f),
                          np.asarray(w_hh_f), np.asarray(b_f),
                          np.asarray(w_ih_b), np.asarray(w_hh_b),
                          np.asarray(b_b), np.asarray(w_tag),
                          np.asarray(b_tag), trans)

    in_maps = []
    for c in range(NCORES):
        m = dict(static)
        pos = np.arange(NPOS, dtype=np.int64) + (SPAN * c - XOFF)
        tokc = np.where((pos >= 0) & (pos < S), sent[np.clip(pos, 0, S - 1)],
                        OOB).astype(np.int32)
        m["tok"] = tokc
        vm = np.zeros((128, WV), np.int32)
        if c == 0:
            for b in range(WV // L):
                vm[b, :WV - L * b] = 1
        m["vmask"] = vm
        in_maps.append(m)

    trace = bool(os.environ.get("BASS_TRACE_KERNEL"))
    if trace:
        import ntff_shim  # noqa: F401
    res = run_bass_kernel_spmd(nc, in_maps, list(range(NCORES)), trace=trace)
    _PROG_CACHE["last_res"] = res

    # host postprocessing: backtrace + path score
    bp = np.zeros((S, T), np.int32)
    feats = np.zeros((S, T), np.float32)
    for c in range(NCORES):
        r = res.results[c]
        bpc = r["bptrs"].reshape(128, L, T)      # [chunk, t, to]
        fvc = r["featsv"].reshape(128, VSTEP, T)[:, WV:, :]
        bp[SPAN * c:SPAN * (c + 1)] = bpc.reshape(SPAN, T)
        feats[SPAN * c:SPAN * (c + 1)] = fvc.reshape(SPAN, T)

    v_end = res.results[NCORES - 1]["vhist"].reshape(128, VSTEP, T)[-1, -1]
    term = (v_end + trans[STOP]).astype(np.float32)
    best = int(np.argmax(term))
    path = np.zeros(S, np.int32)
    tag = best
    for t in range(S - 1, -1, -1):
        path[t] = tag
        tag = bp[t, tag]

    sc = np.float32(0.0)
    prev = START
    for t in range(S):
        sc = np.float32(np.float32(sc + trans[path[t], prev]) + feats[t, path[t]])
        prev = path[t]
    sc = np.float32(sc + trans[STOP, path[-1]])
    return np.float32(sc), path.astype(np.int32)


# revision 20
# speedup vs baseline: 1.1979x; 1.0172x over previous
"""BiLSTM-CRF Viterbi decode on 8 Trainium2 NeuronCores (Bass/Tile).

Strategy (self-contained, shapes hardcoded):
  - 8 cores, core c owns sentence positions [512c, 512(c+1)).
  - Embedding table replicated; each core indirect-DMA-gathers the 640
    token rows covering [512c-64, 512c+576) (out-of-range -> zero rows).
  - pre = x @ w_ih.T + b precomputed on PE for both directions.
  - LSTM recurrence: chunked with zero-init warmup. Per core two
    interleaved streams: forward (128 chunks x 4 tokens, warmup 48) and
    backward (136 chunks covering [512c-32, 512(c+1)), warmup 48).
    Gates are computed on PE (fp32), sigmoid via 0.5*tanh(x/2)+0.5 (the
    Tanh LUT is ~1 ulp), cell update on DVE. Warmup makes each chunk's
    state agree with the sequential scan to ~1 ulp (contraction of the
    LSTM map); boundary cores are exact via zero-x padding.
  - feats = h @ w_tag.T + b_tag computed directly in [chunk, (t, tag)]
    layout (144 small matmuls with column-strided stationary operands).
  - Viterbi: 128 chunks x 4 positions per core, warmup 32 from vinit
    (max-plus coalescence); core-0 short chunks are made exact with
    per-step copy_predicated restores. Backpointers recovered in one
    batched pass (is_equal + reversed-iota + max-reduce).
  - Host: backtrace over device backpointers, score re-accumulated in
    fp32 along the path (bitwise-faithful to the reference recursion).
"""
import os
import sys

for _p in ("/opt/trn_rl_repo", "/root/.axon_site/_ro/trn_rl_repo"):
    if os.path.isdir(_p) and _p not in sys.path:
        sys.path.append(_p)

import numpy as np

import concourse.bass as bass
import concourse.bacc as bacc
import concourse.mybir as mybir
import concourse.tile as tile
from concourse.bass_utils import run_bass_kernel_spmd
from concourse.masks import make_identity

f32 = mybir.dt.float32
i32 = mybir.dt.int32

# problem constants
S = 4096
V = 100000
E = 300
HH = 256
G4 = 1024
T = 24
START, STOP = 22, 23
NEG = -10000.0

NCORES = 8
SPAN = S // NCORES          # 512 positions per core
L = 4                       # tokens per LSTM/viterbi chunk
W = 32                      # LSTM warmup steps
WBF = 20                    # of which: bf16 hh-matmul head
WV = 16                     # viterbi warmup steps
BF = SPAN // L              # 128 forward chunks
BB = (SPAN + WV) // L       # 136 backward chunks (cover 32-pos left spill)
NSTEP = L + W               # 52 LSTM steps per stream
VSTEP = L + WV              # 36 viterbi steps
NPOS = 640                  # gathered x / pre positions: [512c-64, 512c+576)
XOFF = 64                   # position p -> pre column p - 512c + XOFF
HFULL = SPAN + WV           # 544 h columns: [512c-32, 512(c+1))
OOB = 1 << 20

_PROG_CACHE = {}


def _build_program():
    nc = bacc.Bacc("TRN2", target_bir_lowering=False, debug=False,
                   num_devices=NCORES)
    P = 128
    bf16 = mybir.dt.bfloat16

    def din(name, shape, dt=f32):
        return nc.dram_tensor(name, shape, dt, kind="ExternalInput").ap()

    def dout(name, shape, dt=f32):
        return nc.dram_tensor(name, shape, dt, kind="ExternalOutput").ap()

    emb = din("emb", [V, E])
    tok = din("tok", [NPOS], i32)
    wihT = din("wihT", [2, 3, P, G4])
    whhT = din("whhT", [2, 2, P, G4])
    biasr = din("biasr", [P, 16])
    wtagT = din("wtagT", [4, P, T])
    btag_rep = din("btag_rep", [P, VSTEP * T])
    transR = din("transR", [P, T * T])
    vmask = din("vmask", [P, WV], i32)
    iot = din("iot", [P, L * T * T])
    vinit_rep = din("vinit_rep", [P, T])

    o_bptr = dout("bptrs", [P, L * T])
    o_vhist = dout("vhist", [P, VSTEP * T])
    o_feats = dout("featsv", [P, VSTEP * T])

    def bc_mid(ap2, n):
        return bass.AP(ap2.tensor, ap2.offset, [ap2.ap[0], [0, n], ap2.ap[1]])

    def bc_last(ap2, n):
        return bass.AP(ap2.tensor, ap2.offset, [ap2.ap[0], ap2.ap[1], [0, n]])

    with tile.TileContext(nc) as tc:
        with (
            tc.tile_pool(name="const", bufs=1) as cpool,
            tc.tile_pool(name="state", bufs=1) as spool,
        ):
            ident = cpool.tile([P, P], f32)
            make_identity(nc, ident[:])
            ident_bf = cpool.tile([P, P], bf16, name="identbf")
            nc.vector.tensor_copy(ident_bf[:], ident[:])

            hfull = {}
            for d in range(2):
                for k in range(2):
                    for r in range(L):
                        t_ = spool.tile([P, HFULL // L + P], f32,
                                        tag=f"hfull{d}{k}{r}", name=f"hfull{d}{k}{r}")
                        hfull[d, k, r] = t_
            pre_f = spool.tile([P, 8 * NPOS], f32, tag="pref", name="pref")
            pre_pair = {}
            for j in range(4):
                t_ = spool.tile([P, 2 * NPOS], f32, tag=f"pre1{j}", name=f"pre1{j}")
                pre_pair[1, j] = t_
            pre_f_bf = spool.tile([P, 8 * NPOS], bf16, tag="prefbf", name="prefbf")
            pre_bf1 = {}
            for j in range(4):
                t_ = spool.tile([P, 2 * NPOS], bf16, tag=f"preb1{j}", name=f"preb1{j}")
                pre_bf1[j] = t_

            with tc.tile_pool(name="wih", bufs=1) as wpool, \
                    tc.tile_pool(name="psC", bufs=1, space="PSUM") as psC:
                # ---- phase A: gather x, transpose, casts ----
                with tc.tile_pool(name="phA", bufs=1) as apool:
                    tok_sb = apool.tile([P, 5], i32)
                    nc.sync.dma_start(out=tok_sb[:],
                                      in_=tok.rearrange("(j p) -> p j", p=P))
                    xg = []
                    for j in range(5):
                        xt = apool.tile([P, E], f32, tag=f"x{j}", name=f"x{j}")
                        nc.vector.memset(xt[:], 0.0)
                        nc.gpsimd.indirect_dma_start(
                            out=xt[:], out_offset=None, in_=emb[:, :],
                            in_offset=bass.IndirectOffsetOnAxis(
                                ap=tok_sb[:, j:j + 1], axis=0),
                            bounds_check=V - 1, oob_is_err=False)
                        xg.append(xt)
                    xT = []
                    xT_bf = []
                    for k in range(3):
                        t_ = wpool.tile([P, NPOS], f32, tag=f"xT{k}", name=f"xT{k}")
                        if k == 2:
                            nc.vector.memset(t_[:], 0.0)
                        xT.append(t_)
                        tb_ = wpool.tile([P, NPOS], bf16, tag=f"xTb{k}", name=f"xTb{k}")
                        if k == 2:
                            nc.vector.memset(tb_[:], 0)
                        xT_bf.append(tb_)
                    for j in range(5):
                        for k in range(3):
                            ecols = min(128, E - 128 * k)
                            tp = psC.tile([P, P], f32, space="PSUM", tag="pm",
                                          bufs=2, name="tp")
                            nc.tensor.transpose(tp[:ecols, :],
                                                xg[j][:, 128 * k:128 * k + ecols],
                                                ident[:])
                            nc.vector.tensor_copy(
                                xT[k][:ecols, j * P:(j + 1) * P], tp[:ecols, :])
                            nc.vector.tensor_copy(
                                xT_bf[k][:ecols, j * P:(j + 1) * P], tp[:ecols, :])
                    bias_sb = cpool.tile([P, 16], f32)
                    nc.sync.dma_start(out=bias_sb[:], in_=biasr[:])
                    wih_sb = {}
                    wih_bf = {}
                    for d in range(2):
                        for k in range(3):
                            t_ = wpool.tile([P, G4], f32, tag=f"wih{d}{k}", name=f"wih{d}{k}")
                            nc.sync.dma_start(out=t_[:], in_=wihT[d, k])
                            wih_sb[d, k] = t_
                            tb_ = wpool.tile([P, G4], bf16, tag=f"wihb{d}{k}", name=f"wihb{d}{k}")
                            nc.vector.tensor_copy(tb_[:], t_[:])
                            wih_bf[d, k] = tb_

                # ---- phase B-bf16: bf16 pre for the warmup head ----
                def pre_mm(d, m, ncol, c0, use_bf):
                    pm = psC.tile([P, 512], f32, space="PSUM",
                                  tag="pm", bufs=2, name="pm")
                    wsrc = wih_bf if use_bf else wih_sb
                    xsrc = xT_bf if use_bf else xT
                    for k in range(3):
                        nc.tensor.matmul(
                            pm[:, :ncol],
                            wsrc[d, k][:, 128 * m:128 * (m + 1)],
                            xsrc[k][:, c0:c0 + ncol],
                            start=(k == 0), stop=(k == 2))
                    if use_bf:
                        if d == 0:
                            dst = pre_f_bf[:, m * NPOS + c0:m * NPOS + c0 + ncol]
                        else:
                            j, half = divmod(m, 2)
                            dst = pre_bf1[j][:, half * NPOS + c0:
                                             half * NPOS + c0 + ncol]
                    else:
                        if d == 0:
                            dst = pre_f[:, m * NPOS + c0:m * NPOS + c0 + ncol]
                        else:
                            j, half = divmod(m, 2)
                            dst = pre_pair[1, j][:, half * NPOS + c0:
                                                 half * NPOS + c0 + ncol]
                    nc.vector.tensor_scalar_add(
                        dst, pm[:, :ncol], bias_sb[:, d * 8 + m:d * 8 + m + 1])

                for d in range(2):
                    for m in range(8):
                        for ncol, c0 in ((512, 0), (128, 512)):
                            pre_mm(d, m, ncol, c0, True)
                fp32_pre_jobs = [(d, m, ncol, c0)
                                 for d in range(2) for m in range(8)
                                 for ncol, c0 in ((512, 0), (128, 512))]

                whh_sb = {}
                whh_bf = {}
                for d in range(2):
                    for k in range(2):
                        t_ = cpool.tile([P, G4], f32, tag=f"whh{d}{k}", name=f"whh{d}{k}")
                        nc.sync.dma_start(out=t_[:], in_=whhT[d, k])
                        whh_sb[d, k] = t_
                        tb_ = cpool.tile([P, G4], bf16, tag=f"whb{d}{k}", name=f"whb{d}{k}")
                        nc.vector.tensor_copy(tb_[:], t_[:])
                        whh_bf[d, k] = tb_

                # ---- phase C: LSTM streams ----
                streams = [dict(d=0, B=BF), dict(d=1, B=BB)]
                with nc.named_scope("lstm"):
                    for st in streams:
                        B = st["B"]
                        d = st["d"]
                        st["h"] = spool.tile([P, 2 * B], f32, tag=f"h{d}", name=f"hcur{d}")
                        st["c"] = spool.tile([P, 2 * B], f32, tag=f"c{d}", name=f"ccur{d}")
                        st["tmp"] = spool.tile([P, 6 * B], f32, tag=f"tm{d}", name=f"tmp{d}")
                        st["sig"] = spool.tile([P, 6 * B], f32, tag=f"sg{d}", name=f"sig{d}")
                        st["tg"] = spool.tile([P, 2 * B], f32, tag=f"tg{d}", name=f"tgg{d}")
                        st["tc"] = spool.tile([P, 2 * B], f32, tag=f"tc{d}", name=f"tcc{d}")
                        st["pr"] = spool.tile([P, 2 * B], f32, tag=f"pr{d}", name=f"prd{d}")
                        st["hbf"] = spool.tile([P, 2 * B], bf16, tag=f"hb{d}", name=f"hbf{d}")
                        nc.vector.memset(st["h"][:], 0.0)
                        nc.vector.memset(st["c"][:], 0.0)
                        nc.vector.memset(st["hbf"][:], 0)

                    for t in range(NSTEP):
                        for st in streams:
                            d, B = st["d"], st["B"]
                            bf = t < WBF
                            wsel = whh_bf if bf else whh_sb
                            hsel = st["hbf"] if bf else st["h"]
                            if d == 0:
                                poff = (XOFF - W) + t
                                ps = [psC.tile([P, 4 * B], f32, space="PSUM",
                                               tag=f"psf{jj}", name=f"psf{jj}")
                                      for jj in range(2)]
                                for m in range(8):
                                    half, mm = divmod(m, 4)
                                    dstp = ps[half][:, mm * B:(mm + 1) * B]
                                    if bf:
                                        pslice = pre_f_bf[:, m * NPOS + poff::L][:, :B]
                                        nc.tensor.matmul(dstp, ident_bf[:], pslice,
                                                         start=True, stop=False)
                                    for k in range(2):
                                        nc.tensor.matmul(
                                            dstp,
                                            wsel[0, k][:, 128 * m:128 * (m + 1)],
                                            hsel[:, k * B:(k + 1) * B],
                                            start=(k == 0 and not bf),
                                            stop=(k == 1))
                                if not bf:
                                    for half in range(2):
                                        pslice = pre_f[:].rearrange(
                                            "p (m n) -> p m n", m=8)[:, 4 * half:4 * half + 4,
                                                                     poff::L][:, :, :B]
                                        nc.vector.tensor_tensor(
                                            out=ps[half][:].rearrange("p (m b) -> p m b", m=4),
                                            in0=ps[half][:].rearrange("p (m b) -> p m b", m=4),
                                            in1=pslice, op=mybir.AluOpType.add)
                                if bf:
                                    nc.scalar.activation(
                                        st["sig"][:, :4 * B], ps[0][:],
                                        mybir.ActivationFunctionType.Sigmoid)
                                    nc.scalar.activation(
                                        st["sig"][:, 4 * B:6 * B], ps[1][:, :2 * B],
                                        mybir.ActivationFunctionType.Sigmoid)
                                else:
                                    nc.scalar.activation(
                                        st["tmp"][:, :4 * B], ps[0][:],
                                        mybir.ActivationFunctionType.Tanh,
                                        bias=0.0, scale=0.5)
                                    nc.scalar.activation(
                                        st["tmp"][:, 4 * B:6 * B], ps[1][:, :2 * B],
                                        mybir.ActivationFunctionType.Tanh,
                                        bias=0.0, scale=0.5)
                                    nc.scalar.activation(
                                        st["sig"][:], st["tmp"][:],
                                        mybir.ActivationFunctionType.Copy,
                                        bias=0.5, scale=0.5)
                                nc.scalar.activation(
                                    st["tg"][:], ps[1][:, 2 * B:4 * B],
                                    mybir.ActivationFunctionType.Tanh)
                            else:
                                poff = (XOFF - WV) + (L - 1) + W - t
                                ps = [psC.tile([P, 2 * B], f32, space="PSUM",
                                               tag=f"psb{jj}", name=f"psb{jj}")
                                      for jj in range(4)]
                                for j in range(4):
                                    for half in range(2):
                                        m = 2 * j + half
                                        dstp = ps[j][:, half * B:(half + 1) * B]
                                        if bf:
                                            pslice = pre_bf1[j][:, half * NPOS + poff::L][:, :B]
                                            nc.tensor.matmul(dstp, ident_bf[:], pslice,
                                                             start=True, stop=False)
                                        for k in range(2):
                                            nc.tensor.matmul(
                                                dstp,
                                                wsel[1, k][:, 128 * m:128 * (m + 1)],
                                                hsel[:, k * B:(k + 1) * B],
                                                start=(k == 0 and not bf),
                                                stop=(k == 1))
                                if not bf:
                                    for j in range(4):
                                        pslice = pre_pair[1, j][:].rearrange(
                                            "p (m n) -> p m n", m=2)[:, :, poff::L][:, :, :B]
                                        nc.vector.tensor_tensor(
                                            out=ps[j][:].rearrange("p (m b) -> p m b", m=2),
                                            in0=ps[j][:].rearrange("p (m b) -> p m b", m=2),
                                            in1=pslice, op=mybir.AluOpType.add)
                                if bf:
                                    for j in range(3):
                                        nc.scalar.activation(
                                            st["sig"][:, j * 2 * B:(j + 1) * 2 * B],
                                            ps[j][:],
                                            mybir.ActivationFunctionType.Sigmoid)
                                else:
                                    for j in range(3):
                                        nc.scalar.activation(
                                            st["tmp"][:, j * 2 * B:(j + 1) * 2 * B],
                                            ps[j][:],
                                            mybir.ActivationFunctionType.Tanh,
                                            bias=0.0, scale=0.5)
                                    nc.scalar.activation(
                                        st["sig"][:], st["tmp"][:],
                                        mybir.ActivationFunctionType.Copy,
                                        bias=0.5, scale=0.5)
                                nc.scalar.activation(
                                    st["tg"][:], ps[3][:],
                                    mybir.ActivationFunctionType.Tanh)
                            # cell update
                            sig = st["sig"]
                            nc.vector.tensor_tensor(out=st["pr"][:],
                                                    in0=sig[:, :2 * B],
                                                    in1=st["tg"][:],
                                                    op=mybir.AluOpType.mult)
                            nc.vector.tensor_tensor(out=st["c"][:],
                                                    in0=sig[:, 2 * B:4 * B],
                                                    in1=st["c"][:],
                                                    op=mybir.AluOpType.mult)
                            nc.vector.tensor_tensor(out=st["c"][:],
                                                    in0=st["c"][:],
                                                    in1=st["pr"][:],
                                                    op=mybir.AluOpType.add)
                            nc.scalar.activation(st["tc"][:], st["c"][:],
                                                 mybir.ActivationFunctionType.Tanh)
                            hdst = st["hbf"] if t + 1 < WBF else st["h"]
                            nc.vector.tensor_tensor(out=hdst[:],
                                                    in0=sig[:, 4 * B:6 * B],
                                                    in1=st["tc"][:],
                                                    op=mybir.AluOpType.mult)
                            if d == 0:
                                if t >= W:
                                    hoff = t - (W - WV)
                                    r, q = hoff % L, hoff // L
                                    for k in range(2):
                                        nc.vector.tensor_copy(
                                            hfull[0, k, r][:, q:q + B],
                                            st["h"][:, k * B:(k + 1) * B])
                                elif t >= W - WV:
                                    col = t - (W - WV)
                                    r, q = col % L, col // L
                                    hsrc = st["hbf"] if t + 1 < WBF else st["h"]
                                    for k in range(2):
                                        nc.vector.tensor_copy(
                                            hfull[0, k, r][:, q:q + 1],
                                            hsrc[:, k * B:k * B + 1])
                            else:
                                if t >= W:
                                    hoff = (L - 1) - (t - W)
                                    for k in range(2):
                                        nc.vector.tensor_copy(
                                            hfull[1, k, hoff][:, 0:B],
                                            st["h"][:, k * B:(k + 1) * B])
                        # overlap fp32 pre with the bf16 head
                        if t < WBF and fp32_pre_jobs:
                            for _ in range(2):
                                if fp32_pre_jobs:
                                    d_, m_, ncol_, c0_ = fp32_pre_jobs.pop(0)
                                    pre_mm(d_, m_, ncol_, c0_, False)
                    assert not fp32_pre_jobs

            # ---- viterbi constants + pools (after LSTM sbuf freed) ----
            with tc.tile_pool(name="vit", bufs=1) as vpool:
                wtag_sb = []
                for k in range(4):
                    t_ = vpool.tile([P, T], f32, tag=f"wtag{k}", name=f"wtag{k}")
                    nc.sync.dma_start(out=t_[:], in_=wtagT[k])
                    wtag_sb.append(t_)
                btag_sb = vpool.tile([P, VSTEP * T], f32)
                nc.sync.dma_start(out=btag_sb[:], in_=btag_rep[:])
                trR_sb = vpool.tile([P, T * T], f32)
                nc.sync.dma_start(out=trR_sb[:], in_=transR[:])
                vmask_sb = vpool.tile([P, WV], i32)
                nc.sync.dma_start(out=vmask_sb[:], in_=vmask[:])
                iot_sb = vpool.tile([P, L * T * T], f32)
                nc.sync.dma_start(out=iot_sb[:], in_=iot[:])
                vinit_sb = vpool.tile([P, T], f32)
                nc.sync.dma_start(out=vinit_sb[:], in_=vinit_rep[:])

                # ---- phase D+E: feats (PE) pipelined with viterbi (DVE) ----
                featsv = vpool.tile([P, VSTEP * T], f32)
                vhist = vpool.tile([P, VSTEP * T], f32)
                vvhist = vpool.tile([P, L * T], f32)
                schist = vpool.tile([P, L * T * T], f32)
                scscr = vpool.tile([P, T * T], f32)
                vmscr = vpool.tile([P, T], f32)
                trR3 = trR_sb[:].rearrange("p (a b) -> p a b", b=T)
                vprev = vinit_sb[:]
                sc_e = nc.enter_named_scope("viterbi", False)
                with tc.tile_pool(name="psD", bufs=1, space="PSUM") as psD:
                    for t in range(VSTEP):
                        fv = psD.tile([P, T], f32, space="PSUM", tag="fvt",
                                      bufs=4, name="fvt")
                        for k4 in range(4):
                            d, k = divmod(k4, 2)
                            lhs = hfull[d, k, t % L][:, t // L:t // L + P]
                            nc.tensor.matmul(fv[:], lhs, wtag_sb[k4][:],
                                             start=(k4 == 0), stop=(k4 == 3))
                        nc.vector.tensor_tensor(
                            out=featsv[:, t * T:(t + 1) * T], in0=fv[:],
                            in1=btag_sb[:, t * T:(t + 1) * T],
                            op=mybir.AluOpType.add)
                        real = t >= WV
                        sc_ap = (schist[:, (t - WV) * T * T:(t - WV + 1) * T * T]
                                 if real else scscr[:])
                        sc3 = sc_ap.rearrange("p (a b) -> p a b", b=T)
                        nc.vector.tensor_tensor(out=sc3, in0=bc_mid(vprev, T),
                                                in1=trR3, op=mybir.AluOpType.add)
                        vv_ap = (vvhist[:, (t - WV) * T:(t - WV + 1) * T]
                                 if real else vmscr[:])
                        nc.vector.tensor_reduce(out=vv_ap, in_=sc3,
                                                axis=mybir.AxisListType.X,
                                                op=mybir.AluOpType.max)
                        vdst = vhist[:, t * T:(t + 1) * T]
                        nc.vector.tensor_tensor(out=vdst, in0=vv_ap,
                                                in1=featsv[:, t * T:(t + 1) * T],
                                                op=mybir.AluOpType.add)
                        if t < WV:
                            mb = vmask_sb[:, t:t + 1]
                            mask_bc = bass.AP(mb.tensor, mb.offset,
                                              [mb.ap[0], [0, T]])
                            nc.vector.copy_predicated(vdst, mask_bc, vprev)
                        vprev = vdst
                nc.sync.dma_start(out=o_feats[:], in_=featsv[:])
                # ---- phase F: backpointers ----
                mask = vpool.tile([P, L * T * T], f32)
                sch3 = schist[:].rearrange("p (a b) -> p a b", b=T)
                vvb = bc_last(vvhist[:], T)
                nc.vector.tensor_tensor(out=mask[:].rearrange("p (a b) -> p a b", b=T),
                                        in0=sch3, in1=vvb,
                                        op=mybir.AluOpType.is_equal)
                nc.vector.tensor_tensor(out=mask[:], in0=mask[:], in1=iot_sb[:],
                                        op=mybir.AluOpType.mult)
                r96 = vpool.tile([P, L * T], f32)
                nc.vector.tensor_reduce(out=r96[:],
                                        in_=mask[:].rearrange("p (a b) -> p a b", b=T),
                                        axis=mybir.AxisListType.X,
                                        op=mybir.AluOpType.max)
                bp_sb = vpool.tile([P, L * T], f32)
                nc.scalar.activation(bp_sb[:], r96[:],
                                     mybir.ActivationFunctionType.Copy,
                                     bias=float(T), scale=-1.0)
                nc.sync.dma_start(out=o_bptr[:], in_=bp_sb[:])
                nc.sync.dma_start(out=o_vhist[:], in_=vhist[:])
                nc.leave_named_scope("viterbi", sc_e[0], False)
    nc.compile()
    return nc


def _prep_static(emb_table, w_ih_f, w_hh_f, b_f, w_ih_b, w_hh_b, b_b,
                 w_tag, b_tag, transitions):
    """Host-side weight reordering/padding shared by all cores."""
    P = 128
    perm = np.r_[0:256, 256:512, 768:1024, 512:768]  # [i, f, o, g]
    out = {}
    out["emb"] = np.ascontiguousarray(emb_table.astype(np.float32))

    wihT = np.zeros((2, 3, P, G4), np.float32)
    whhT = np.zeros((2, 2, P, G4), np.float32)
    biasr = np.zeros((P, 16), np.float32)
    for d, (wi, wh, bb_) in enumerate(((w_ih_f, w_hh_f, b_f),
                                       (w_ih_b, w_hh_b, b_b))):
        wiT = wi[perm].T.astype(np.float32)          # [300, 1024]
        wiTp = np.zeros((384, G4), np.float32)
        wiTp[:E] = wiT
        for k in range(3):
            wihT[d, k] = wiTp[128 * k:128 * (k + 1)]
        whT = wh[perm].T.astype(np.float32)          # [256, 1024]
        for k in range(2):
            whhT[d, k] = whT[128 * k:128 * (k + 1)]
        biasr[:, d * 8:(d + 1) * 8] = bb_[perm].astype(np.float32).reshape(8, P).T
    out["wihT"], out["whhT"], out["biasr"] = wihT, whhT, biasr

    wtT = w_tag.T.astype(np.float32)                 # [512, 24]
    out["wtagT"] = wtT.reshape(4, P, T).copy()

    out["btag_rep"] = np.tile(b_tag.astype(np.float32)[None, :],
                              (P, VSTEP)).reshape(P, VSTEP * T).copy()
    out["transR"] = np.tile(transitions.astype(np.float32).reshape(1, T * T),
                            (P, 1)).copy()
    iot = np.tile((float(T) - np.arange(T, dtype=np.float32))[None, None, :],
                  (P, L * T, 1)).reshape(P, L * T * T)
    out["iot"] = np.ascontiguousarray(iot)
    vinit = np.full(T, NEG, np.float32)
    vinit[START] = 0.0
    out["vinit_rep"] = np.tile(vinit[None, :], (P, 1)).copy()
    return out


def kernel(sentence, emb_table, w_ih_f, w_hh_f, b_f, w_ih_b, w_hh_b, b_b,
           w_tag, b_tag, transitions):
    sentence = np.asarray(sentence)
    sent = sentence.astype(np.int64)
    trans = np.asarray(transitions, np.float32)

    if "nc" not in _PROG_CACHE:
        _PROG_CACHE["nc"] = _build_program()
    nc = _PROG_CACHE["nc"]

    static = _prep_static(np.asarray(emb_table), np.asarray(w_ih_f),
                          np.asarray(w_hh_f), np.asarray(b_f),
                          np.asarray(w_ih_b), np.asarray(w_hh_b),
                          np.asarray(b_b), np.asarray(w_tag),
                          np.asarray(b_tag), trans)

    in_maps = []
    for c in range(NCORES):
        m = dict(static)
        pos = np.arange(NPOS, dtype=np.int64) + (SPAN * c - XOFF)
        tokc = np.where((pos >= 0) & (pos < S), sent[np.clip(pos, 0, S - 1)],
                        OOB).astype(np.int32)
        m["tok"] = tokc
        vm = np.zeros((128, WV), np.int32)
        if c == 0:
            for b in range(WV // L):
                vm[b, :WV - L * b] = 1
        m["vmask"] = vm
        in_maps.append(m)

    trace = bool(os.environ.get("BASS_TRACE_KERNEL"))
    if trace:
        import ntff_shim  # noqa: F401
    res = run_bass_kernel_spmd(nc, in_maps, list(range(NCORES)), trace=trace)
    _PROG_CACHE["last_res"] = res

    # host postprocessing: backtrace + path score
    bp = np.zeros((S, T), np.int32)
    feats = np.zeros((S, T), np.float32)
    for c in range(NCORES):
        r = res.results[c]
        bpc = r["bptrs"].reshape(128, L, T)      # [chunk, t, to]
        fvc = r["featsv"].reshape(128, VSTEP, T)[:, WV:, :]
        bp[SPAN * c:SPAN * (c + 1)] = bpc.reshape(SPAN, T)
        feats[SPAN * c:SPAN * (c + 1)] = fvc.reshape(SPAN, T)

    v_end = res.results[NCORES - 1]["vhist"].reshape(128, VSTEP, T)[-1, -1]
    term = (v_end + trans[STOP]).astype(np.float32)
    best = int(np.argmax(term))
    path = np.zeros(S, np.int32)
    tag = best
    for t in range(S - 1, -1, -1):
        path[t] = tag
        tag = bp[t, tag]

    sc = np.float32(0.0)
    prev = START
    for t in range(S):
        sc = np.float32(np.float32(sc + trans[path[t], prev]) + feats[t, path[t]])
        prev = path[t]
    sc = np.float32(sc + trans[STOP, path[-1]])
    return np.float32(sc), path.astype(np.int32)


# revision 21
# speedup vs baseline: 1.1995x; 1.0013x over previous
"""BiLSTM-CRF Viterbi decode on 8 Trainium2 NeuronCores (Bass/Tile).

Strategy (self-contained, shapes hardcoded):
  - 8 cores, core c owns sentence positions [512c, 512(c+1)).
  - Embedding table replicated; each core indirect-DMA-gathers the 640
    token rows covering [512c-64, 512c+576) (out-of-range -> zero rows).
  - pre = x @ w_ih.T + b precomputed on PE for both directions.
  - LSTM recurrence: chunked with zero-init warmup. Per core two
    interleaved streams: forward (128 chunks x 4 tokens, warmup 48) and
    backward (136 chunks covering [512c-32, 512(c+1)), warmup 48).
    Gates are computed on PE (fp32), sigmoid via 0.5*tanh(x/2)+0.5 (the
    Tanh LUT is ~1 ulp), cell update on DVE. Warmup makes each chunk's
    state agree with the sequential scan to ~1 ulp (contraction of the
    LSTM map); boundary cores are exact via zero-x padding.
  - feats = h @ w_tag.T + b_tag computed directly in [chunk, (t, tag)]
    layout (144 small matmuls with column-strided stationary operands).
  - Viterbi: 128 chunks x 4 positions per core, warmup 32 from vinit
    (max-plus coalescence); core-0 short chunks are made exact with
    per-step copy_predicated restores. Backpointers recovered in one
    batched pass (is_equal + reversed-iota + max-reduce).
  - Host: backtrace over device backpointers, score re-accumulated in
    fp32 along the path (bitwise-faithful to the reference recursion).
"""
import os
import sys

for _p in ("/opt/trn_rl_repo", "/root/.axon_site/_ro/trn_rl_repo"):
    if os.path.isdir(_p) and _p not in sys.path:
        sys.path.append(_p)

import numpy as np

import concourse.bass as bass
import concourse.bacc as bacc
import concourse.mybir as mybir
import concourse.tile as tile
from concourse.bass_utils import run_bass_kernel_spmd
from concourse.masks import make_identity

f32 = mybir.dt.float32
i32 = mybir.dt.int32

# problem constants
S = 4096
V = 100000
E = 300
HH = 256
G4 = 1024
T = 24
START, STOP = 22, 23
NEG = -10000.0

NCORES = 8
SPAN = S // NCORES          # 512 positions per core
L = 4                       # tokens per LSTM/viterbi chunk
W = 32                      # LSTM warmup steps
WBF = 20                    # of which: bf16 hh-matmul head
WV = 16                     # viterbi warmup steps
BF = SPAN // L              # 128 forward chunks
BB = (SPAN + WV) // L       # 136 backward chunks (cover 32-pos left spill)
NSTEP = L + W               # 52 LSTM steps per stream
VSTEP = L + WV              # 36 viterbi steps
NPOS = 640                  # gathered x / pre positions: [512c-64, 512c+576)
XOFF = 64                   # position p -> pre column p - 512c + XOFF
HFULL = SPAN + WV           # 544 h columns: [512c-32, 512(c+1))
OOB = 1 << 20

_PROG_CACHE = {}


def _build_program():
    nc = bacc.Bacc("TRN2", target_bir_lowering=False, debug=False,
                   num_devices=NCORES)
    P = 128
    bf16 = mybir.dt.bfloat16

    def din(name, shape, dt=f32):
        return nc.dram_tensor(name, shape, dt, kind="ExternalInput").ap()

    def dout(name, shape, dt=f32):
        return nc.dram_tensor(name, shape, dt, kind="ExternalOutput").ap()

    emb = din("emb", [V, E])
    tok = din("tok", [NPOS], i32)
    wihT = din("wihT", [2, 3, P, G4])
    whhT = din("whhT", [2, 2, P, G4])
    biasr = din("biasr", [P, 16])
    wtagT = din("wtagT", [4, P, T])
    btag_rep = din("btag_rep", [P, VSTEP * T])
    transR = din("transR", [P, T * T])
    vmask = din("vmask", [P, WV], i32)
    iot = din("iot", [P, L * T * T])
    vinit_rep = din("vinit_rep", [P, T])

    o_bptr = dout("bptrs", [P, L * T])
    o_vhist = dout("vhist", [P, VSTEP * T])
    o_feats = dout("featsv", [P, VSTEP * T])

    def bc_mid(ap2, n):
        return bass.AP(ap2.tensor, ap2.offset, [ap2.ap[0], [0, n], ap2.ap[1]])

    def bc_last(ap2, n):
        return bass.AP(ap2.tensor, ap2.offset, [ap2.ap[0], ap2.ap[1], [0, n]])

    with tile.TileContext(nc) as tc:
        with (
            tc.tile_pool(name="const", bufs=1) as cpool,
            tc.tile_pool(name="state", bufs=1) as spool,
        ):
            ident = cpool.tile([P, P], f32)
            make_identity(nc, ident[:])
            ident_bf = cpool.tile([P, P], bf16, name="identbf")
            nc.vector.tensor_copy(ident_bf[:], ident[:])

            hfull = {}
            for d in range(2):
                for k in range(2):
                    for r in range(L):
                        t_ = spool.tile([P, HFULL // L + P], f32,
                                        tag=f"hfull{d}{k}{r}", name=f"hfull{d}{k}{r}")
                        hfull[d, k, r] = t_
            pre_f = spool.tile([P, 8 * NPOS], f32, tag="pref", name="pref")
            pre_pair = {}
            for j in range(4):
                t_ = spool.tile([P, 2 * NPOS], f32, tag=f"pre1{j}", name=f"pre1{j}")
                pre_pair[1, j] = t_
            pre_f_bf = spool.tile([P, 8 * NPOS], bf16, tag="prefbf", name="prefbf")
            pre_bf1 = {}
            for j in range(4):
                t_ = spool.tile([P, 2 * NPOS], bf16, tag=f"preb1{j}", name=f"preb1{j}")
                pre_bf1[j] = t_

            with tc.tile_pool(name="wih", bufs=1) as wpool, \
                    tc.tile_pool(name="psC", bufs=1, space="PSUM") as psC:
                # ---- phase A: gather x, transpose, casts ----
                with tc.tile_pool(name="phA", bufs=1) as apool:
                    tok_sb = apool.tile([P, 5], i32)
                    nc.sync.dma_start(out=tok_sb[:],
                                      in_=tok.rearrange("(j p) -> p j", p=P))
                    xg = []
                    for j in range(5):
                        xt = apool.tile([P, E], f32, tag=f"x{j}", name=f"x{j}")
                        nc.vector.memset(xt[:], 0.0)
                        nc.gpsimd.indirect_dma_start(
                            out=xt[:], out_offset=None, in_=emb[:, :],
                            in_offset=bass.IndirectOffsetOnAxis(
                                ap=tok_sb[:, j:j + 1], axis=0),
                            bounds_check=V - 1, oob_is_err=False)
                        xg.append(xt)
                    xT = []
                    xT_bf = []
                    for k in range(3):
                        t_ = wpool.tile([P, NPOS], f32, tag=f"xT{k}", name=f"xT{k}")
                        if k == 2:
                            nc.vector.memset(t_[:], 0.0)
                        xT.append(t_)
                        tb_ = wpool.tile([P, NPOS], bf16, tag=f"xTb{k}", name=f"xTb{k}")
                        if k == 2:
                            nc.vector.memset(tb_[:], 0)
                        xT_bf.append(tb_)
                    for j in range(5):
                        for k in range(3):
                            ecols = min(128, E - 128 * k)
                            tp = psC.tile([P, P], f32, space="PSUM", tag="pm",
                                          bufs=2, name="tp")
                            nc.tensor.transpose(tp[:ecols, :],
                                                xg[j][:, 128 * k:128 * k + ecols],
                                                ident[:])
                            nc.vector.tensor_copy(
                                xT[k][:ecols, j * P:(j + 1) * P], tp[:ecols, :])
                            nc.vector.tensor_copy(
                                xT_bf[k][:ecols, j * P:(j + 1) * P], tp[:ecols, :])
                    bias_sb = cpool.tile([P, 16], f32)
                    nc.sync.dma_start(out=bias_sb[:], in_=biasr[:])
                    wih_sb = {}
                    wih_bf = {}
                    for d in range(2):
                        for k in range(3):
                            t_ = wpool.tile([P, G4], f32, tag=f"wih{d}{k}", name=f"wih{d}{k}")
                            nc.sync.dma_start(out=t_[:], in_=wihT[d, k])
                            wih_sb[d, k] = t_
                            tb_ = wpool.tile([P, G4], bf16, tag=f"wihb{d}{k}", name=f"wihb{d}{k}")
                            nc.vector.tensor_copy(tb_[:], t_[:])
                            wih_bf[d, k] = tb_

                # ---- phase B-bf16: bf16 pre for the warmup head ----
                def pre_mm(d, m, ncol, c0, use_bf):
                    pm = psC.tile([P, 512], f32, space="PSUM",
                                  tag="pm", bufs=2, name="pm")
                    wsrc = wih_bf if use_bf else wih_sb
                    xsrc = xT_bf if use_bf else xT
                    for k in range(3):
                        nc.tensor.matmul(
                            pm[:, :ncol],
                            wsrc[d, k][:, 128 * m:128 * (m + 1)],
                            xsrc[k][:, c0:c0 + ncol],
                            start=(k == 0), stop=(k == 2))
                    if use_bf:
                        if d == 0:
                            dst = pre_f_bf[:, m * NPOS + c0:m * NPOS + c0 + ncol]
                        else:
                            j, half = divmod(m, 2)
                            dst = pre_bf1[j][:, half * NPOS + c0:
                                             half * NPOS + c0 + ncol]
                    else:
                        if d == 0:
                            dst = pre_f[:, m * NPOS + c0:m * NPOS + c0 + ncol]
                        else:
                            j, half = divmod(m, 2)
                            dst = pre_pair[1, j][:, half * NPOS + c0:
                                                 half * NPOS + c0 + ncol]
                    nc.vector.tensor_scalar_add(
                        dst, pm[:, :ncol], bias_sb[:, d * 8 + m:d * 8 + m + 1])

                for d in range(2):
                    for m in range(8):
                        for ncol, c0 in ((512, 0), (128, 512)):
                            pre_mm(d, m, ncol, c0, True)
                fp32_pre_jobs = [(d, m, ncol, c0)
                                 for d in range(2) for m in range(8)
                                 for ncol, c0 in ((512, 0), (128, 512))]

                whh_sb = {}
                whh_bf = {}
                for d in range(2):
                    for k in range(2):
                        t_ = cpool.tile([P, G4], f32, tag=f"whh{d}{k}", name=f"whh{d}{k}")
                        nc.sync.dma_start(out=t_[:], in_=whhT[d, k])
                        whh_sb[d, k] = t_
                        tb_ = cpool.tile([P, G4], bf16, tag=f"whb{d}{k}", name=f"whb{d}{k}")
                        nc.vector.tensor_copy(tb_[:], t_[:])
                        whh_bf[d, k] = tb_

                # ---- phase C: LSTM streams ----
                streams = [dict(d=0, B=BF), dict(d=1, B=BB)]
                with nc.named_scope("lstm"):
                    for st in streams:
                        B = st["B"]
                        d = st["d"]
                        st["h"] = spool.tile([P, 2 * B], f32, tag=f"h{d}", name=f"hcur{d}")
                        st["c"] = spool.tile([P, 2 * B], f32, tag=f"c{d}", name=f"ccur{d}")
                        st["tmp"] = spool.tile([P, 6 * B], f32, tag=f"tm{d}", name=f"tmp{d}")
                        st["sig"] = spool.tile([P, 6 * B], f32, tag=f"sg{d}", name=f"sig{d}")
                        st["tg"] = spool.tile([P, 2 * B], f32, tag=f"tg{d}", name=f"tgg{d}")
                        st["tc"] = spool.tile([P, 2 * B], f32, tag=f"tc{d}", name=f"tcc{d}")
                        st["pr"] = spool.tile([P, 2 * B], f32, tag=f"pr{d}", name=f"prd{d}")
                        st["hbf"] = spool.tile([P, 2 * B], bf16, tag=f"hb{d}", name=f"hbf{d}")
                        nc.vector.memset(st["h"][:], 0.0)
                        nc.vector.memset(st["c"][:], 0.0)
                        nc.vector.memset(st["hbf"][:], 0)

                    for t in range(NSTEP):
                        for st in streams:
                            d, B = st["d"], st["B"]
                            bf = t < WBF
                            wsel = whh_bf if bf else whh_sb
                            hsel = st["hbf"] if bf else st["h"]
                            if d == 0:
                                poff = (XOFF - W) + t
                                ps = [psC.tile([P, 4 * B], f32, space="PSUM",
                                               tag=f"psf{jj}", name=f"psf{jj}")
                                      for jj in range(2)]
                                for m in range(8):
                                    half, mm = divmod(m, 4)
                                    dstp = ps[half][:, mm * B:(mm + 1) * B]
                                    for k in range(2):
                                        nc.tensor.matmul(
                                            dstp,
                                            wsel[0, k][:, 128 * m:128 * (m + 1)],
                                            hsel[:, k * B:(k + 1) * B],
                                            start=(k == 0), stop=(k == 1))
                                psrc = pre_f_bf if bf else pre_f
                                for half in range(2):
                                    pslice = psrc[:].rearrange(
                                        "p (m n) -> p m n", m=8)[:, 4 * half:4 * half + 4,
                                                                 poff::L][:, :, :B]
                                    nc.vector.tensor_tensor(
                                        out=ps[half][:].rearrange("p (m b) -> p m b", m=4),
                                        in0=ps[half][:].rearrange("p (m b) -> p m b", m=4),
                                        in1=pslice, op=mybir.AluOpType.add)
                                if bf:
                                    nc.scalar.activation(
                                        st["sig"][:, :4 * B], ps[0][:],
                                        mybir.ActivationFunctionType.Sigmoid)
                                    nc.scalar.activation(
                                        st["sig"][:, 4 * B:6 * B], ps[1][:, :2 * B],
                                        mybir.ActivationFunctionType.Sigmoid)
                                else:
                                    nc.scalar.activation(
                                        st["tmp"][:, :4 * B], ps[0][:],
                                        mybir.ActivationFunctionType.Tanh,
                                        bias=0.0, scale=0.5)
                                    nc.scalar.activation(
                                        st["tmp"][:, 4 * B:6 * B], ps[1][:, :2 * B],
                                        mybir.ActivationFunctionType.Tanh,
                                        bias=0.0, scale=0.5)
                                    nc.scalar.activation(
                                        st["sig"][:], st["tmp"][:],
                                        mybir.ActivationFunctionType.Copy,
                                        bias=0.5, scale=0.5)
                                nc.scalar.activation(
                                    st["tg"][:], ps[1][:, 2 * B:4 * B],
                                    mybir.ActivationFunctionType.Tanh)
                            else:
                                poff = (XOFF - WV) + (L - 1) + W - t
                                ps = [psC.tile([P, 2 * B], f32, space="PSUM",
                                               tag=f"psb{jj}", name=f"psb{jj}")
                                      for jj in range(4)]
                                for j in range(4):
                                    for half in range(2):
                                        m = 2 * j + half
                                        dstp = ps[j][:, half * B:(half + 1) * B]
                                        for k in range(2):
                                            nc.tensor.matmul(
                                                dstp,
                                                wsel[1, k][:, 128 * m:128 * (m + 1)],
                                                hsel[:, k * B:(k + 1) * B],
                                                start=(k == 0), stop=(k == 1))
                                for j in range(4):
                                    psrc = pre_bf1[j] if bf else pre_pair[1, j]
                                    pslice = psrc[:].rearrange(
                                        "p (m n) -> p m n", m=2)[:, :, poff::L][:, :, :B]
                                    nc.vector.tensor_tensor(
                                        out=ps[j][:].rearrange("p (m b) -> p m b", m=2),
                                        in0=ps[j][:].rearrange("p (m b) -> p m b", m=2),
                                        in1=pslice, op=mybir.AluOpType.add)
                                if bf:
                                    for j in range(3):
                                        nc.scalar.activation(
                                            st["sig"][:, j * 2 * B:(j + 1) * 2 * B],
                                            ps[j][:],
                                            mybir.ActivationFunctionType.Sigmoid)
                                else:
                                    for j in range(3):
                                        nc.scalar.activation(
                                            st["tmp"][:, j * 2 * B:(j + 1) * 2 * B],
                                            ps[j][:],
                                            mybir.ActivationFunctionType.Tanh,
                                            bias=0.0, scale=0.5)
                                    nc.scalar.activation(
                                        st["sig"][:], st["tmp"][:],
                                        mybir.ActivationFunctionType.Copy,
                                        bias=0.5, scale=0.5)
                                nc.scalar.activation(
                                    st["tg"][:], ps[3][:],
                                    mybir.ActivationFunctionType.Tanh)
                            # cell update
                            sig = st["sig"]
                            nc.vector.tensor_tensor(out=st["pr"][:],
                                                    in0=sig[:, :2 * B],
                                                    in1=st["tg"][:],
                                                    op=mybir.AluOpType.mult)
                            nc.vector.tensor_tensor(out=st["c"][:],
                                                    in0=sig[:, 2 * B:4 * B],
                                                    in1=st["c"][:],
                                                    op=mybir.AluOpType.mult)
                            nc.vector.tensor_tensor(out=st["c"][:],
                                                    in0=st["c"][:],
                                                    in1=st["pr"][:],
                                                    op=mybir.AluOpType.add)
                            nc.scalar.activation(st["tc"][:], st["c"][:],
                                                 mybir.ActivationFunctionType.Tanh)
                            hdst = st["hbf"] if t + 1 < WBF else st["h"]
                            nc.vector.tensor_tensor(out=hdst[:],
                                                    in0=sig[:, 4 * B:6 * B],
                                                    in1=st["tc"][:],
                                                    op=mybir.AluOpType.mult)
                            if d == 0:
                                if t >= W:
                                    hoff = t - (W - WV)
                                    r, q = hoff % L, hoff // L
                                    for k in range(2):
                                        nc.vector.tensor_copy(
                                            hfull[0, k, r][:, q:q + B],
                                            st["h"][:, k * B:(k + 1) * B])
                                elif t >= W - WV:
                                    col = t - (W - WV)
                                    r, q = col % L, col // L
                                    hsrc = st["hbf"] if t + 1 < WBF else st["h"]
                                    for k in range(2):
                                        nc.vector.tensor_copy(
                                            hfull[0, k, r][:, q:q + 1],
                                            hsrc[:, k * B:k * B + 1])
                            else:
                                if t >= W:
                                    hoff = (L - 1) - (t - W)
                                    for k in range(2):
                                        nc.vector.tensor_copy(
                                            hfull[1, k, hoff][:, 0:B],
                                            st["h"][:, k * B:(k + 1) * B])
                        # overlap fp32 pre with the bf16 head
                        if t < WBF and fp32_pre_jobs:
                            for _ in range(2):
                                if fp32_pre_jobs:
                                    d_, m_, ncol_, c0_ = fp32_pre_jobs.pop(0)
                                    pre_mm(d_, m_, ncol_, c0_, False)
                    assert not fp32_pre_jobs

            # ---- viterbi constants + pools (after LSTM sbuf freed) ----
            with tc.tile_pool(name="vit", bufs=1) as vpool:
                wtag_sb = []
                for k in range(4):
                    t_ = vpool.tile([P, T], f32, tag=f"wtag{k}", name=f"wtag{k}")
                    nc.sync.dma_start(out=t_[:], in_=wtagT[k])
                    wtag_sb.append(t_)
                btag_sb = vpool.tile([P, VSTEP * T], f32)
                nc.sync.dma_start(out=btag_sb[:], in_=btag_rep[:])
                trR_sb = vpool.tile([P, T * T], f32)
                nc.sync.dma_start(out=trR_sb[:], in_=transR[:])
                vmask_sb = vpool.tile([P, WV], i32)
                nc.sync.dma_start(out=vmask_sb[:], in_=vmask[:])
                iot_sb = vpool.tile([P, L * T * T], f32)
                nc.sync.dma_start(out=iot_sb[:], in_=iot[:])
                vinit_sb = vpool.tile([P, T], f32)
                nc.sync.dma_start(out=vinit_sb[:], in_=vinit_rep[:])

                # ---- phase D+E: feats (PE) pipelined with viterbi (DVE) ----
                featsv = vpool.tile([P, VSTEP * T], f32)
                vhist = vpool.tile([P, VSTEP * T], f32)
                vvhist = vpool.tile([P, L * T], f32)
                schist = vpool.tile([P, L * T * T], f32)
                scscr = vpool.tile([P, T * T], f32)
                vmscr = vpool.tile([P, T], f32)
                trR3 = trR_sb[:].rearrange("p (a b) -> p a b", b=T)
                vprev = vinit_sb[:]
                sc_e = nc.enter_named_scope("viterbi", False)
                with tc.tile_pool(name="psD", bufs=1, space="PSUM") as psD:
                    for t in range(VSTEP):
                        fv = psD.tile([P, T], f32, space="PSUM", tag="fvt",
                                      bufs=4, name="fvt")
                        for k4 in range(4):
                            d, k = divmod(k4, 2)
                            lhs = hfull[d, k, t % L][:, t // L:t // L + P]
                            nc.tensor.matmul(fv[:], lhs, wtag_sb[k4][:],
                                             start=(k4 == 0), stop=(k4 == 3))
                        nc.vector.tensor_tensor(
                            out=featsv[:, t * T:(t + 1) * T], in0=fv[:],
                            in1=btag_sb[:, t * T:(t + 1) * T],
                            op=mybir.AluOpType.add)
                        real = t >= WV
                        sc_ap = (schist[:, (t - WV) * T * T:(t - WV + 1) * T * T]
                                 if real else scscr[:])
                        sc3 = sc_ap.rearrange("p (a b) -> p a b", b=T)
                        nc.vector.tensor_tensor(out=sc3, in0=bc_mid(vprev, T),
                                                in1=trR3, op=mybir.AluOpType.add)
                        vv_ap = (vvhist[:, (t - WV) * T:(t - WV + 1) * T]
                                 if real else vmscr[:])
                        nc.vector.tensor_reduce(out=vv_ap, in_=sc3,
                                                axis=mybir.AxisListType.X,
                                                op=mybir.AluOpType.max)
                        vdst = vhist[:, t * T:(t + 1) * T]
                        nc.vector.tensor_tensor(out=vdst, in0=vv_ap,
                                                in1=featsv[:, t * T:(t + 1) * T],
                                                op=mybir.AluOpType.add)
                        if t < WV:
                            mb = vmask_sb[:, t:t + 1]
                            mask_bc = bass.AP(mb.tensor, mb.offset,
                                              [mb.ap[0], [0, T]])
                            nc.vector.copy_predicated(vdst, mask_bc, vprev)
                        vprev = vdst
                nc.sync.dma_start(out=o_feats[:], in_=featsv[:])
                # ---- phase F: backpointers ----
                mask = vpool.tile([P, L * T * T], f32)
                sch3 = schist[:].rearrange("p (a b) -> p a b", b=T)
                vvb = bc_last(vvhist[:], T)
                nc.vector.tensor_tensor(out=mask[:].rearrange("p (a b) -> p a b", b=T),
                                        in0=sch3, in1=vvb,
                                        op=mybir.AluOpType.is_equal)
                nc.vector.tensor_tensor(out=mask[:], in0=mask[:], in1=iot_sb[:],
                                        op=mybir.AluOpType.mult)
                r96 = vpool.tile([P, L * T], f32)
                nc.vector.tensor_reduce(out=r96[:],
                                        in_=mask[:].rearrange("p (a b) -> p a b", b=T),
                                        axis=mybir.AxisListType.X,
                                        op=mybir.AluOpType.max)
                bp_sb = vpool.tile([P, L * T], f32)
                nc.scalar.activation(bp_sb[:], r96[:],
                                     mybir.ActivationFunctionType.Copy,
                                     bias=float(T), scale=-1.0)
                nc.sync.dma_start(out=o_bptr[:], in_=bp_sb[:])
                nc.sync.dma_start(out=o_vhist[:], in_=vhist[:])
                nc.leave_named_scope("viterbi", sc_e[0], False)
    nc.compile()
    return nc


def _prep_static(emb_table, w_ih_f, w_hh_f, b_f, w_ih_b, w_hh_b, b_b,
                 w_tag, b_tag, transitions):
    """Host-side weight reordering/padding shared by all cores."""
    P = 128
    perm = np.r_[0:256, 256:512, 768:1024, 512:768]  # [i, f, o, g]
    out = {}
    out["emb"] = np.ascontiguousarray(emb_table.astype(np.float32))

    wihT = np.zeros((2, 3, P, G4), np.float32)
    whhT = np.zeros((2, 2, P, G4), np.float32)
    biasr = np.zeros((P, 16), np.float32)
    for d, (wi, wh, bb_) in enumerate(((w_ih_f, w_hh_f, b_f),
                                       (w_ih_b, w_hh_b, b_b))):
        wiT = wi[perm].T.astype(np.float32)          # [300, 1024]
        wiTp = np.zeros((384, G4), np.float32)
        wiTp[:E] = wiT
        for k in range(3):
            wihT[d, k] = wiTp[128 * k:128 * (k + 1)]
        whT = wh[perm].T.astype(np.float32)          # [256, 1024]
        for k in range(2):
            whhT[d, k] = whT[128 * k:128 * (k + 1)]
        biasr[:, d * 8:(d + 1) * 8] = bb_[perm].astype(np.float32).reshape(8, P).T
    out["wihT"], out["whhT"], out["biasr"] = wihT, whhT, biasr

    wtT = w_tag.T.astype(np.float32)                 # [512, 24]
    out["wtagT"] = wtT.reshape(4, P, T).copy()

    out["btag_rep"] = np.tile(b_tag.astype(np.float32)[None, :],
                              (P, VSTEP)).reshape(P, VSTEP * T).copy()
    out["transR"] = np.tile(transitions.astype(np.float32).reshape(1, T * T),
                            (P, 1)).copy()
    iot = np.tile((float(T) - np.arange(T, dtype=np.float32))[None, None, :],
                  (P, L * T, 1)).reshape(P, L * T * T)
    out["iot"] = np.ascontiguousarray(iot)
    vinit = np.full(T, NEG, np.float32)
    vinit[START] = 0.0
    out["vinit_rep"] = np.tile(vinit[None, :], (P, 1)).copy()
    return out


def kernel(sentence, emb_table, w_ih_f, w_hh_f, b_f, w_ih_b, w_hh_b, b_b,
           w_tag, b_tag, transitions):
    sentence = np.asarray(sentence)
    sent = sentence.astype(np.int64)
    trans = np.asarray(transitions, np.float32)

    if "nc" not in _PROG_CACHE:
        _PROG_CACHE["nc"] = _build_program()
    nc = _PROG_CACHE["nc"]

    static = _prep_static(np.asarray(emb_table), np.asarray(w_ih_f),
                          np.asarray(w_hh_f), np.asarray(b_f),
                          np.asarray(w_ih_b), np.asarray(w_hh_b),
                          np.asarray(b_b), np.asarray(w_tag),
                          np.asarray(b_tag), trans)

    in_maps = []
    for c in range(NCORES):
        m = dict(static)
        pos = np.arange(NPOS, dtype=np.int64) + (SPAN * c - XOFF)
        tokc = np.where((pos >= 0) & (pos < S), sent[np.clip(pos, 0, S - 1)],
                        OOB).astype(np.int32)
        m["tok"] = tokc
        vm = np.zeros((128, WV), np.int32)
        if c == 0:
            for b in range(WV // L):
                vm[b, :WV - L * b] = 1
        m["vmask"] = vm
        in_maps.append(m)

    trace = bool(os.environ.get("BASS_TRACE_KERNEL"))
    if trace:
        import ntff_shim  # noqa: F401
    res = run_bass_kernel_spmd(nc, in_maps, list(range(NCORES)), trace=trace)
    _PROG_CACHE["last_res"] = res

    # host postprocessing: backtrace + path score
    bp = np.zeros((S, T), np.int32)
    feats = np.zeros((S, T), np.float32)
    for c in range(NCORES):
        r = res.results[c]
        bpc = r["bptrs"].reshape(128, L, T)      # [chunk, t, to]
        fvc = r["featsv"].reshape(128, VSTEP, T)[:, WV:, :]
        bp[SPAN * c:SPAN * (c + 1)] = bpc.reshape(SPAN, T)
        feats[SPAN * c:SPAN * (c + 1)] = fvc.reshape(SPAN, T)

    v_end = res.results[NCORES - 1]["vhist"].reshape(128, VSTEP, T)[-1, -1]
    term = (v_end + trans[STOP]).astype(np.float32)
    best = int(np.argmax(term))
    path = np.zeros(S, np.int32)
    tag = best
    for t in range(S - 1, -1, -1):
        path[t] = tag
        tag = bp[t, tag]

    sc = np.float32(0.0)
    prev = START
    for t in range(S):
        sc = np.float32(np.float32(sc + trans[path[t], prev]) + feats[t, path[t]])
        prev = path[t]
    sc = np.float32(sc + trans[STOP, path[-1]])
    return np.float32(sc), path.astype(np.int32)
